# revision 20
# baseline (speedup 1.0000x reference)
"""2-layer GCNConv (PyG-style, normalize=True) on 8 Trainium2 NeuronCores.

Strategy (graph/data parallel, per sharding hint):
- Nodes sharded 8 ways (core c owns rows [c*6250, (c+1)*6250)); edges
  partitioned by destination-node owner.
- Weights replicated. Layer-1 dense transform (h1 = x @ W1) is computed
  REPLICATED on every core over all 50000 nodes in bf16 (PE time is
  trivial), eliminating the layer-1 AllGather; h1 is written to two local
  DRAM bf16 half-tables (src < 25000 / >= 25000) so per-edge gathers for
  half 0 start while half 1 is still being written.
- Per-edge source features fetched with dma_gather (SWDGE gather ucode);
  segment-sum by destination done on the TensorEngine as matmuls against
  host-built segment matrices S (norm coefficients baked in), accumulated
  in PSUM. Self-loop terms are folded in as extra edges with norm = 1/deg.
- Schedule: per-core balanced node->tile permutation (greedy bin-pack by
  per-half in-degree) with SHARED per-(tile,half) bucket capacities, and
  128-slot groups that span tile boundaries (matmul per (group, tile)
  segment) -- cuts slot padding from +41% to +9%. The permutation is
  undone on host after the run.
- Aggregation runs as two passes (half 0 then half 1): each pass gathers
  and accumulates its own PSUM per superblock, combined into the output
  SBUF tile by DVE (write+bias, then add), decoupling half-0 progress
  from half-1 table availability.
- Layer 2: h2 = out1 @ W2 on own rows only (f32), one AllGather builds
  the gatherable f32 h2 table (rows = permuted node order, so the L2
  gather uses its own idx table).
- deg/norm/schedule are integer-graph-structure preprocessing on host.

Layer 1 aggregation is computed transposed (gathered rows stationary,
S moving) so its output [96, own] directly feeds layer 2's dense matmul
as the stationary operand. Layer 2 aggregation is computed row-major
(S stationary) so the final output lands in row layout.
"""

import numpy as np

# problem constants (hardcoded per contract)
N_NODES = 50000
N_CORES = 8
OWN = N_NODES // N_CORES          # 6250
F_IN = 96
F_H1 = 96                         # layer-1 output width
F_H2 = 64                         # layer-2 output width
ROW1 = 128                        # h1-table row: 128 bf16 = 256 B
HALF = 25000                      # int16 gather index split point
TN = 32                           # nodes per segment tile (PSUM cols per matmul)
TPB = 16                          # node-tiles per superblock (1 PSUM bank for L1)
GSZ = 128                         # edge slots per group (= contraction dim)
CALL_MAX_GROUPS = 8               # max groups per dma_gather call (1024 slots; SWDGE ring limit)

N_TILES = (OWN + TN - 1) // TN            # 196
N_SB = (N_TILES + TPB - 1) // TPB         # 13
CAP_LAST = OWN - (N_TILES - 1) * TN       # 10
_DBG_SKIP = set()  # debug: subset of {'ag2','agg1','agg2','gather','mm'}
_DMA_SCRATCH = 65536              # SWDGE ring (SBUF B/partition): 4096 descs
GDST_BUFS = 6
N_QUEUES = 1
XCHUNK = 2048                     # xT streaming chunk (dense-1 phase)
DBATCH = 4                        # dense-1 node blocks batched per PSUM bank


def _balance(d0, d1):
    """Greedy assign OWN nodes (per-half loads d0,d1) into N_TILES tiles of
    <=TN nodes (last tile CAP_LAST), minimizing per-tile per-half max load."""
    caps = np.full(N_TILES, TN, np.int64)
    caps[-1] = CAP_LAST
    order = np.argsort(-(d0 + d1), kind="stable")
    l0 = np.zeros(N_TILES)
    l1 = np.zeros(N_TILES)
    used = np.zeros(N_TILES, np.int64)
    tile_of = np.empty(OWN, np.int64)
    dloc = np.empty(OWN, np.int64)
    for n in order:
        score = np.maximum(l0 + d0[n], l1 + d1[n])
        score[used >= caps] = np.inf
        t = int(np.argmin(score))
        tile_of[n] = t
        dloc[n] = used[t]
        used[t] += 1
        l0[t] += d0[n]
        l1[t] += d1[n]
    return tile_of, dloc


def _preprocess(src, dst):
    """Host-side integer-structure preprocessing -> shared schedule +
    per-core idx/S arrays."""
    deg = np.bincount(dst, minlength=N_NODES).astype(np.float64) + 1.0
    dinv = (1.0 / np.sqrt(deg)).astype(np.float32)

    # fold self-loops in as edges
    arange_n = np.arange(N_NODES, dtype=np.int64)
    src_a = np.concatenate([src, arange_n])
    dst_a = np.concatenate([dst, arange_n])
    norm_a = np.concatenate([
        dinv[src] * dinv[dst],
        (dinv * dinv).astype(np.float32),
    ]).astype(np.float32)

    owner = dst_a // OWN
    half = (src_a >= HALF).astype(np.int64)

    # per-dst-node per-half in-edge counts (self loops folded in)
    nodecnt = np.bincount(dst_a * 2 + half, minlength=N_NODES * 2).reshape(
        N_NODES, 2
    )

    # per-core balanced permutation
    tile_of = np.empty(N_NODES, np.int64)
    dloc = np.empty(N_NODES, np.int64)
    for c in range(N_CORES):
        lo, hi = c * OWN, (c + 1) * OWN
        t_c, d_c = _balance(nodecnt[lo:hi, 0].astype(np.float64),
                            nodecnt[lo:hi, 1].astype(np.float64))
        tile_of[lo:hi] = t_c
        dloc[lo:hi] = d_c

    p_local = tile_of * TN + dloc            # permuted local row, < OWN
    row_l2 = (np.arange(N_NODES) // OWN) * OWN + p_local
    perm_global = np.empty(N_NODES, np.int64)
    perm_global[row_l2] = np.arange(N_NODES)  # device row -> natural node

    # shared bucket capacities: C[t,h] = max over cores of bucket count
    t_e = tile_of[dst_a]
    d_e = dloc[dst_a]
    key = (owner * N_TILES + t_e) * 2 + half
    cnt = np.bincount(key, minlength=N_CORES * N_TILES * 2).reshape(
        N_CORES, N_TILES, 2
    )
    C = cnt.max(axis=0)                      # [T, 2]

    # slot layout: runs (b, h); tiles sequential, run padded to GSZ multiple
    tile_slot_base = np.zeros((N_TILES, 2), np.int64)
    runs = []          # (b, h, g0, g1, s0, s1)  group + segment ranges
    calls = []         # (b, h, g0, g1)
    segments = []      # (g, tl_local, b, t_global)
    slot_cursor = 0
    group_cursor = 0
    for b in range(N_SB):
        t_lo = b * TPB
        t_hi = min(t_lo + TPB, N_TILES)
        for h in (0, 1):
            run_s0 = slot_cursor
            run_g0 = group_cursor
            seg_s0 = len(segments)
            for t in range(t_lo, t_hi):
                tile_slot_base[t, h] = slot_cursor
                slot_cursor += int(C[t, h])
            run_slots = slot_cursor - run_s0
            run_slots_p = -(-run_slots // GSZ) * GSZ
            slot_cursor = run_s0 + run_slots_p
            n_g = run_slots_p // GSZ
            group_cursor += n_g
            g = run_g0
            while g < group_cursor:
                g1 = min(g + CALL_MAX_GROUPS, group_cursor)
                calls.append((b, h, g, g1))
                g = g1
            for gl in range(n_g):
                s0 = run_s0 + gl * GSZ
                s1 = s0 + GSZ
                for t in range(t_lo, t_hi):
                    tb = tile_slot_base[t, h]
                    te = tb + int(C[t, h])
                    if tb < s1 and te > s0:
                        segments.append((run_g0 + gl, t - t_lo, b, t))
            runs.append((b, h, run_g0, group_cursor, seg_s0, len(segments)))
    n_groups = group_cursor
    n_segs = len(segments)
    tot_slots = n_groups * GSZ

    # vectorized (group, tile) -> segment lookup
    seg_arr = np.full(n_groups * N_TILES, -1, np.int64)
    for s, (g, tl, b, t) in enumerate(segments):
        seg_arr[g * N_TILES + t] = s

    # per-core slot assignment
    idxA = np.zeros((N_CORES, tot_slots), np.int16)
    idxB = np.zeros((N_CORES, tot_slots), np.int16)
    S_val = np.zeros((N_CORES, GSZ, n_segs * TN), np.float32)
    for c in range(N_CORES):
        m = owner == c
        s_c = src_a[m]
        t_c = t_e[m]
        h_c = half[m]
        n_c = norm_a[m]
        d_c = d_e[m]
        keyth = t_c * 2 + h_c
        order = np.argsort(keyth, kind="stable")
        k_s = keyth[order]
        start_of = np.searchsorted(k_s, np.arange(N_TILES * 2))
        ranks = np.arange(k_s.size) - start_of[k_s]
        slots = tile_slot_base.reshape(-1)[k_s] + ranks
        g_of = slots // GSZ
        seg_of = seg_arr[g_of * N_TILES + t_c[order]]
        idxA[c, slots] = (s_c[order] - h_c[order] * HALF).astype(np.int16)
        rb = row_l2[s_c[order]]
        idxB[c, slots] = (rb - h_c[order] * HALF).astype(np.int16)
        S_val[c, slots % GSZ, seg_of * TN + d_c[order]] = n_c[order]

    def wrap(idx):
        out = np.empty((N_CORES, 128, tot_slots // 16), np.int16)
        for c in range(N_CORES):
            a = idx[c].reshape(tot_slots // 16, 16).T
            out[c] = np.tile(a, (8, 1))
        return out

    sched = {
        "runs": runs,
        "calls": calls,
        "segments": segments,
        "n_groups": n_groups,
        "n_segs": n_segs,
        "perm_global": perm_global,
    }
    return sched, (wrap(idxA), wrap(idxB)), S_val


def _build_program(sched, repeat=1, repeat_phase="all"):
    import concourse.bacc as bacc
    import concourse.mybir as mybir
    import concourse.tile as tile

    runs = sched["runs"]
    calls = sched["calls"]
    segments = sched["segments"]
    n_groups = sched["n_groups"]
    n_segs = sched["n_segs"]
    tot_slots = n_groups * GSZ
    f32 = mybir.dt.float32
    bf16 = mybir.dt.bfloat16

    nc = bacc.Bacc("TRN2", target_bir_lowering=False, debug=False,
                   num_devices=N_CORES,
                   dynamic_dma_scratch_size=_DMA_SCRATCH,
                   num_swdge_queues=N_QUEUES)

    d_xTf = nc.dram_tensor("xTf", [F_IN, N_NODES], bf16, kind="ExternalInput")
    d_W1 = nc.dram_tensor("W1", [F_IN, F_H1], bf16, kind="ExternalInput")
    d_W2 = nc.dram_tensor("W2", [F_H1, F_H2], f32, kind="ExternalInput")
    d_b1 = nc.dram_tensor("b1c", [F_H1, 1], f32, kind="ExternalInput")
    d_b2 = nc.dram_tensor("b2r", [128, 4 * F_H2], f32, kind="ExternalInput")
    d_S1 = nc.dram_tensor("S1", [128, n_segs * TN], bf16, kind="ExternalInput")
    d_idxA = nc.dram_tensor("idxA", [128, tot_slots // 16], mybir.dt.int16,
                            kind="ExternalInput")
    d_idxB = nc.dram_tensor("idxB", [128, tot_slots // 16], mybir.dt.int16,
                            kind="ExternalInput")
    d_out = nc.dram_tensor("out", [OWN, F_H2], f32, kind="ExternalOutput")

    rg = [list(range(N_CORES))]

    with tile.TileContext(nc) as tc:
        with (
            tc.tile_pool(name="sb", bufs=1) as sb,
            tc.tile_pool(name="sbx", bufs=2) as sbx,
            tc.tile_pool(name="sbg", bufs=GDST_BUFS) as sbg,
            tc.tile_pool(name="sbs", bufs=4) as sbs,
            tc.tile_pool(name="sbh", bufs=2) as sbh,
            tc.tile_pool(name="ps", bufs=2, space="PSUM") as ps,
            tc.tile_pool(name="dram", bufs=1, space="DRAM") as dram,
        ):
            # --- resident tiles ---
            W1_t = sb.tile([F_IN, F_H1], bf16)
            nc.sync.dma_start(W1_t[:], d_W1[:])
            W2_t = sb.tile([F_H1, F_H2], f32)
            nc.sync.dma_start(W2_t[:], d_W2[:])
            b1_t = sb.tile([F_H1, 1], f32)
            nc.sync.dma_start(b1_t[:], d_b1[:])
            b2_t = sb.tile([128, 4 * F_H2], f32)
            nc.sync.dma_start(b2_t[:], d_b2[:])
            idxA_t = sb.tile([128, tot_slots // 16], mybir.dt.int16)
            nc.sync.dma_start(idxA_t[:], d_idxA[:])
            idxB_t = sb.tile([128, tot_slots // 16], mybir.dt.int16)
            nc.sync.dma_start(idxB_t[:], d_idxB[:])
            out1T = sb.tile([F_H1, N_TILES * TN], f32)
            out2 = sb.tile([128, ((OWN + 127) // 128) * F_H2, ], f32)

            # two gatherable h1 half-tables (bf16, 256B rows); separate tiles
            # so gathers of half A don't wait on half-B writes
            h1_tabA = dram.tile([HALF, ROW1], bf16)
            h1_tabB = dram.tile([HALF, ROW1], bf16)
            h2_shard = dram.tile([OWN, F_H2], f32)
            h2_table = dram.tile([N_NODES, F_H2], f32)

            def _phase_reps(name):
                if repeat_phase == "all" or repeat_phase == name:
                    return repeat
                return 1

            for _rep in range(repeat if repeat_phase == "all" else 1):
                # --- phase A: full replicated h1 = x @ W1 -> half tables ---
                for hh, o in [(hh, o) for _ in range(_phase_reps("denseA"))
                              for hh in (0, 1)
                              for o in range(0, HALF, XCHUNK)]:
                    w = min(XCHUNK, HALF - o)
                    base = hh * HALF + o            # global node base
                    nblk = (w + 127) // 128
                    wp = nblk * 128                 # block-padded width
                    xc = sbx.tile([F_IN, XCHUNK], bf16, tag="xc",
                                  name=f"xc_{base}")
                    nc.sync.dma_start(xc[:, :w], d_xTf[:, base:base + w])
                    if wp > w:                      # zero-pad the tail block
                        nc.vector.memset(xc[:, w:wp], 0.0)
                    stg = sbh.tile([128, (XCHUNK // 128) * ROW1], bf16,
                                   tag="h1stg", name=f"stg_{base}")
                    tab = h1_tabA if hh == 0 else h1_tabB
                    # zero-fill the 96:128 pad cols the copies skip
                    nc.vector.memset(
                        stg[:, 0:nblk * ROW1].rearrange(
                            "p (c f) -> p c f", f=ROW1)[:, :, F_H1:], 0.0)
                    for j0 in range(0, nblk, DBATCH):
                        j1 = min(j0 + DBATCH, nblk)
                        p_d = ps.tile([128, DBATCH * F_H1], f32, tag="dense",
                                      name=f"pd1_{base}_{j0}")
                        for k in range(j0, j1):
                            nc.tensor.matmul(
                                out=p_d[:, (k - j0) * F_H1:(k - j0 + 1) * F_H1],
                                lhsT=xc[:, k * 128:(k + 1) * 128],
                                rhs=W1_t[:],
                                start=(k == j0), stop=(k == j1 - 1),
                            )
                        nc.scalar.copy(
                            out=stg[:, j0 * ROW1:j1 * ROW1].rearrange(
                                "p (c f) -> p c f", f=ROW1)[:, :, 0:F_H1],
                            in_=p_d[:, 0:(j1 - j0) * F_H1].rearrange(
                                "p (c f) -> p c f", f=F_H1),
                        )
                    wfull = (w // 128) * 128
                    if wfull:
                        nc.sync.dma_start(
                            tab[o:o + wfull, :].rearrange(
                                "(c p) f -> p c f", p=128),
                            stg[:, 0:(wfull // 128) * ROW1].rearrange(
                                "p (c f) -> p c f", f=ROW1),
                        )
                    if w - wfull:
                        nc.sync.dma_start(
                            tab[o + wfull:o + w, :],
                            stg[:w - wfull,
                                (wfull // 128) * ROW1:(wfull // 128 + 1) * ROW1],
                        )

                # map group index -> (call index, slot within call)
                call_of_group = {}
                for ci, (b, h, g0, g1) in enumerate(calls):
                    for g in range(g0, g1):
                        call_of_group[g] = (ci, g - g0)

                def agg_layer(tag, tabs, idx_t, d_S, s_dt, elem, alloc_cb,
                              mm_cb, out_cb, bank_of, convert=False):
                    """Gather + segment-matmul driver, two passes by half.

                    tabs = (half0_table_ap, half1_table_ap)
                    alloc_cb(b, h) -> psum tile for (superblock, pass)
                    mm_cb(p_t, gt, goff, S_t, seg_local, tl, (start, stop))
                    out_cb(b, psum_tile, w_tiles, h)  # h=0 write, h=1 accum
                    bank_of(tl) -> psum zero-region id for start/stop flags
                    convert: downcast gathered f32 tiles to bf16 (ACT copy)
                    """
                    call_tiles = [None] * len(calls)
                    order = [ci for ci, c in enumerate(calls) if c[1] == 0] + \
                            [ci for ci, c in enumerate(calls) if c[1] == 1]
                    for ci in order:
                        b, h, g0, g1 = calls[ci]
                        nblk = g1 - g0
                        gt = sbg.tile([128, nblk * elem], tabs[h].dtype,
                                      tag="gdst", name=f"g_{tag}_{g0}")
                        nslots = nblk * GSZ
                        if 'gather' not in _DBG_SKIP:
                            nc.gpsimd.dma_gather(
                                out_ap=gt[:].rearrange("p (b e) -> p b e",
                                                       e=elem),
                                in_ap=tabs[h],
                                idxs_ap=idx_t[:, g0 * GSZ // 16:
                                              g1 * GSZ // 16],
                                num_idxs=nslots,
                                num_idxs_reg=nslots,
                                elem_size=elem,
                                elem_step=elem,
                                queue_num=ci % N_QUEUES,
                            )
                        if convert:
                            gtb = sbg.tile([128, nblk * elem], bf16,
                                           tag="gconv", name=f"gb_{tag}_{g0}")
                            nc.scalar.copy(out=gtb[:], in_=gt[:])
                            call_tiles[ci] = gtb
                        else:
                            call_tiles[ci] = gt
                    # per-(pass, superblock): S chunk + segment matmuls
                    for hp in (0, 1):
                        for b, h, g0b, g1b, s0, s1 in runs:
                            if h != hp or s0 == s1:
                                continue
                            w_tiles = min(TPB, N_TILES - b * TPB)
                            S_t = sbs.tile([128, (s1 - s0) * TN], s_dt,
                                           tag="sseg",
                                           name=f"s_{tag}_{b}_{h}")
                            nc.sync.dma_start(S_t[:],
                                              d_S[:, s0 * TN:s1 * TN])
                            p_t = alloc_cb(b, h)
                            first_in_bank, last_in_bank = {}, {}
                            for si in range(s0, s1):
                                bk = bank_of(segments[si][1])
                                if bk not in first_in_bank:
                                    first_in_bank[bk] = si
                                last_in_bank[bk] = si
                            if 'mm' not in _DBG_SKIP:
                                for si in range(s0, s1):
                                    g, tl = segments[si][0], segments[si][1]
                                    bk = bank_of(tl)
                                    ci, goff = call_of_group[g]
                                    mm_cb(p_t, call_tiles[ci], goff, S_t,
                                          si - s0, tl,
                                          (first_in_bank[bk] == si,
                                           last_in_bank[bk] == si))
                                out_cb(b, p_t, w_tiles, hp)

                # L1 callbacks: h-stationary -> psum [96, TPB*TN]
                def l1_alloc(b, h):
                    return ps.tile([F_H1, TPB * TN], f32, tag="agg1",
                                   name=f"pa1_{b}_{h}")

                def l1_mm(p_t, gt, goff, S_t, sl, tl, flags):
                    start, stop = flags
                    nc.tensor.matmul(
                        out=p_t[:, tl * TN:(tl + 1) * TN],
                        lhsT=gt[:, goff * ROW1: goff * ROW1 + F_H1],
                        rhs=S_t[:, sl * TN:(sl + 1) * TN],
                        start=start, stop=stop,
                    )

                def l1_out(b, p_t, w_tiles, h):
                    w = w_tiles * TN
                    dst = out1T[:, b * TPB * TN: b * TPB * TN + w]
                    if h == 0:
                        nc.vector.tensor_scalar(
                            out=dst, in0=p_t[:, :w],
                            scalar1=b1_t[:, 0:1], scalar2=None,
                            op0=mybir.AluOpType.add,
                        )
                    else:
                        nc.vector.tensor_tensor(
                            out=dst, in0=dst, in1=p_t[:, :w],
                            op=mybir.AluOpType.add,
                        )

                if 'mm' in _DBG_SKIP:
                    nc.vector.memset(out1T[:], 0.0)
                if 'agg1' not in _DBG_SKIP:
                    for _pr in range(_phase_reps("agg1")):
                        agg_layer("l1", (h1_tabA[:], h1_tabB[:]), idxA_t,
                                  d_S1, bf16, ROW1,
                                  l1_alloc, l1_mm, l1_out, lambda tl: 0)
                else:
                    nc.vector.memset(out1T[:], 0.0)

                # --- phase D: h2_own = out1 @ W2 -> h2_shard ---
                for r0 in [r for _ in range(_phase_reps("denseD"))
                           for r in range(0, OWN, 1024)]:
                    w = min(1024, OWN - r0)
                    nblk = (w + 127) // 128
                    stg2 = sbh.tile([128, 8 * F_H2], f32, tag="h2stg",
                                    name=f"stg2_{r0}")
                    p_d = ps.tile([128, 8 * F_H2], f32, tag="dense",
                                  name=f"pd2_{r0}")
                    for k in range(nblk):
                        # out1T is N_TILES*TN = 6272 wide, so the tail block
                        # can read a full 128 cols (tile-195 pad cols)
                        nc.tensor.matmul(
                            out=p_d[:, k * F_H2:(k + 1) * F_H2],
                            lhsT=out1T[:, r0 + k * 128:r0 + (k + 1) * 128],
                            rhs=W2_t[:],
                            start=(k == 0), stop=(k == nblk - 1),
                        )
                    nc.scalar.copy(out=stg2[:, :nblk * F_H2],
                                   in_=p_d[:, :nblk * F_H2])
                    wfull = (w // 128) * 128
                    if wfull:
                        nc.sync.dma_start(
                            h2_shard[r0:r0 + wfull, :].rearrange(
                                "(c p) f -> p c f", p=128),
                            stg2[:, 0:(wfull // 128) * F_H2].rearrange(
                                "p (c f) -> p c f", f=F_H2),
                        )
                    if w - wfull:
                        nc.sync.dma_start(
                            h2_shard[r0 + wfull:r0 + w, :],
                            stg2[:w - wfull,
                                 (wfull // 128) * F_H2:(wfull // 128 + 1) * F_H2],
                        )

                # --- phase E: AllGather h2 ---
                if 'ag2' not in _DBG_SKIP:
                  for _pr in range(_phase_reps("ag2")):
                    nc.gpsimd.collective_compute(
                        "AllGather", mybir.AluOpType.bypass, replica_groups=rg,
                        ins=[h2_shard.opt()], outs=[h2_table.opt()],
                    )
                else:
                    nc.sync.dma_start(h2_table[0:OWN, :], h2_shard[:])

                # --- phase F: layer-2 aggregation (row-major out) ---
                def l2_alloc(b, h):
                    return ps.tile([TN, TPB * F_H2], f32, tag="agg2",
                                   name=f"pa2_{b}_{h}")

                def l2_mm(p_t, gt, goff, S_t, sl, tl, flags):
                    start, stop = flags
                    nc.tensor.matmul(
                        out=p_t[:, tl * F_H2:(tl + 1) * F_H2],
                        lhsT=S_t[:, sl * TN:(sl + 1) * TN],
                        rhs=gt[:, goff * F_H2:(goff + 1) * F_H2],
                        start=start, stop=stop,
                    )

                def l2_out(b, p_t, w_tiles, h):
                    # psum [32, tl*64] ; node n = b*TPB*TN + tl*TN + j
                    # -> out2 partition 32*(tl%4)+j, chunk 4*b + tl//4
                    for q in range(min(4, w_tiles)):
                        # tiles tl = 4*c' + q for c' in range(n_q)
                        n_q = (w_tiles - q + 3) // 4
                        src = p_t[:, q * F_H2:].rearrange(
                            "p (c f) -> p c f", f=F_H2)[:, 0:4 * (n_q - 1) + 1:4, :]
                        dstp = out2[q * TN:(q + 1) * TN,
                                    (4 * b) * F_H2:(4 * b + n_q) * F_H2]
                        dstr = dstp.rearrange("p (c f) -> p c f", f=F_H2)
                        if h == 0:
                            nc.vector.tensor_tensor(
                                out=dstr, in0=src,
                                in1=b2_t[q * TN:(q + 1) * TN,
                                         :n_q * F_H2].rearrange(
                                    "p (c f) -> p c f", f=F_H2),
                                op=mybir.AluOpType.add,
                            )
                        else:
                            nc.vector.tensor_tensor(
                                out=dstr, in0=dstr, in1=src,
                                op=mybir.AluOpType.add,
                            )

                if 'mm' in _DBG_SKIP:
                    nc.vector.memset(out2[:], 0.0)
                if 'agg2' not in _DBG_SKIP:
                    for _pr in range(_phase_reps("agg2")):
                        agg_layer("l2", (h2_table[:HALF, :], h2_table[HALF:, :]),
                                  idxB_t, d_S1, bf16, F_H2,
                                  l2_alloc, l2_mm, l2_out,
                                  lambda tl: tl * F_H2 * 4 // 2048,
                                  convert=True)
                else:
                    nc.vector.memset(out2[:], 0.0)

                # --- final output ---
                full = (OWN // 128) * 128        # 6144
                nc.sync.dma_start(
                    d_out[0:full, :].rearrange("(c p) f -> p c f", p=128),
                    out2[:, 0:(full // 128) * F_H2].rearrange(
                        "p (c f) -> p c f", f=F_H2),
                )
                rem = OWN - full
                if rem:
                    nc.sync.dma_start(
                        d_out[full:OWN, :],
                        out2[:rem, (full // 128) * F_H2:(full // 128 + 1) * F_H2],
                    )

    nc.compile()
    return nc


def _make_in_maps(x, W1, b1, W2, b2, S_all, idx_wrapped):
    import ml_dtypes

    idxA, idxB = idx_wrapped
    xTf = np.ascontiguousarray(
        np.asarray(x, np.float32).T.astype(ml_dtypes.bfloat16))
    b2r = np.ascontiguousarray(np.tile(np.asarray(b2, np.float32)[None, :],
                                       (128, 4)))
    in_maps = []
    for c in range(N_CORES):
        in_maps.append({
            "xTf": xTf,
            "W1": np.asarray(W1, np.float32).astype(ml_dtypes.bfloat16),
            "W2": np.asarray(W2, np.float32),
            "b1c": np.ascontiguousarray(np.asarray(b1, np.float32)[:, None]),
            "b2r": b2r,
            "S1": np.ascontiguousarray(S_all[c].astype(ml_dtypes.bfloat16)),
            "idxA": np.ascontiguousarray(idxA[c]),
            "idxB": np.ascontiguousarray(idxB[c]),
        })
    return in_maps


def _postprocess(out, sched):
    res = np.empty_like(out)
    res[sched["perm_global"]] = out
    return res


def kernel(x, edge_index, W1, b1, W2, b2):
    from concourse.bass_utils import run_bass_kernel_spmd

    ei = np.asarray(edge_index)
    src = ei[0].astype(np.int64)
    dst = ei[1].astype(np.int64)

    sched, idx_wrapped, S_all = _preprocess(src, dst)
    nc = _build_program(sched)

    in_maps = _make_in_maps(x, W1, b1, W2, b2, S_all, idx_wrapped)
    res = run_bass_kernel_spmd(nc, in_maps, core_ids=list(range(N_CORES)))
    out = np.concatenate([res.results[c]["out"] for c in range(N_CORES)], axis=0)
    return _postprocess(out.astype(np.float32), sched)


# revision 21
# speedup vs baseline: 1.0959x; 1.0959x over previous
"""2-layer GCNConv (PyG-style, normalize=True) on 8 Trainium2 NeuronCores.

Strategy (graph/data parallel, per sharding hint):
- Nodes sharded 8 ways (core c owns rows [c*6250, (c+1)*6250)); edges
  partitioned by destination-node owner.
- Weights replicated. Layer-1 dense transform (h1 = x @ W1) is computed
  REPLICATED on every core over all 50000 nodes in bf16 (PE time is
  trivial), eliminating the layer-1 AllGather; h1 is written to two local
  DRAM bf16 half-tables (src < 25000 / >= 25000) so per-edge gathers for
  half 0 start while half 1 is still being written.
- Per-edge source features fetched with dma_gather (SWDGE gather ucode);
  segment-sum by destination done on the TensorEngine as matmuls against
  host-built segment matrices S (norm coefficients baked in), accumulated
  in PSUM. Self-loop terms are folded in as extra edges with norm = 1/deg.
- Schedule: per-core balanced node->tile permutation (greedy bin-pack by
  per-half in-degree) with SHARED per-(tile,half) bucket capacities, and
  128-slot groups that span tile boundaries (matmul per (group, tile)
  segment) -- cuts slot padding from +41% to +9%. The permutation is
  undone on host after the run.
- Aggregation runs as two passes (half 0 then half 1): each pass gathers
  and accumulates its own PSUM per superblock, combined into the output
  SBUF tile by DVE (write+bias, then add), decoupling half-0 progress
  from half-1 table availability.
- Layer 2: h2 = out1 @ W2 on own rows only (f32), one AllGather builds
  the gatherable f32 h2 table (rows = permuted node order, so the L2
  gather uses its own idx table).
- deg/norm/schedule are integer-graph-structure preprocessing on host.

Layer 1 aggregation is computed transposed (gathered rows stationary,
S moving) so its output [96, own] directly feeds layer 2's dense matmul
as the stationary operand. Layer 2 aggregation is computed row-major
(S stationary) so the final output lands in row layout.
"""

import numpy as np

# problem constants (hardcoded per contract)
N_NODES = 50000
N_CORES = 8
OWN = N_NODES // N_CORES          # 6250
F_IN = 96
F_H1 = 96                         # layer-1 output width
F_H2 = 64                         # layer-2 output width
ROW1 = 128                        # h1-table row: 128 bf16 = 256 B
HALF = 25000                      # int16 gather index split point
TN = 32                           # nodes per segment tile (PSUM cols per matmul)
TPB = 16                          # node-tiles per superblock (1 PSUM bank for L1)
GSZ = 128                         # edge slots per group (= contraction dim)
CALL_MAX_GROUPS = 8               # max groups per dma_gather call (1024 slots; SWDGE ring limit)

N_TILES = (OWN + TN - 1) // TN            # 196
N_SB = (N_TILES + TPB - 1) // TPB         # 13
CAP_LAST = OWN - (N_TILES - 1) * TN       # 10
_DBG_SKIP = set()  # debug: subset of {'ag2','agg1','agg2','gather','mm'}
_DMA_SCRATCH = 65536              # SWDGE ring (SBUF B/partition): 4096 descs
GDST_BUFS = 6
N_QUEUES = 2
XCHUNK = 2048                     # xT streaming chunk (dense-1 phase)
DBATCH = 4                        # dense-1 node blocks batched per PSUM bank


def _balance(d0, d1):
    """Greedy assign OWN nodes (per-half loads d0,d1) into N_TILES tiles of
    <=TN nodes (last tile CAP_LAST), minimizing per-tile per-half max load."""
    caps = np.full(N_TILES, TN, np.int64)
    caps[-1] = CAP_LAST
    order = np.argsort(-(d0 + d1), kind="stable")
    l0 = np.zeros(N_TILES)
    l1 = np.zeros(N_TILES)
    used = np.zeros(N_TILES, np.int64)
    tile_of = np.empty(OWN, np.int64)
    dloc = np.empty(OWN, np.int64)
    for n in order:
        score = np.maximum(l0 + d0[n], l1 + d1[n])
        score[used >= caps] = np.inf
        t = int(np.argmin(score))
        tile_of[n] = t
        dloc[n] = used[t]
        used[t] += 1
        l0[t] += d0[n]
        l1[t] += d1[n]
    return tile_of, dloc


def _preprocess(src, dst):
    """Host-side integer-structure preprocessing -> shared schedule +
    per-core idx/S arrays."""
    deg = np.bincount(dst, minlength=N_NODES).astype(np.float64) + 1.0
    dinv = (1.0 / np.sqrt(deg)).astype(np.float32)

    # fold self-loops in as edges
    arange_n = np.arange(N_NODES, dtype=np.int64)
    src_a = np.concatenate([src, arange_n])
    dst_a = np.concatenate([dst, arange_n])
    norm_a = np.concatenate([
        dinv[src] * dinv[dst],
        (dinv * dinv).astype(np.float32),
    ]).astype(np.float32)

    owner = dst_a // OWN
    half = (src_a >= HALF).astype(np.int64)

    # per-dst-node per-half in-edge counts (self loops folded in)
    nodecnt = np.bincount(dst_a * 2 + half, minlength=N_NODES * 2).reshape(
        N_NODES, 2
    )

    # per-core balanced permutation
    tile_of = np.empty(N_NODES, np.int64)
    dloc = np.empty(N_NODES, np.int64)
    for c in range(N_CORES):
        lo, hi = c * OWN, (c + 1) * OWN
        t_c, d_c = _balance(nodecnt[lo:hi, 0].astype(np.float64),
                            nodecnt[lo:hi, 1].astype(np.float64))
        tile_of[lo:hi] = t_c
        dloc[lo:hi] = d_c

    p_local = tile_of * TN + dloc            # permuted local row, < OWN
    row_l2 = (np.arange(N_NODES) // OWN) * OWN + p_local
    perm_global = np.empty(N_NODES, np.int64)
    perm_global[row_l2] = np.arange(N_NODES)  # device row -> natural node

    # shared bucket capacities: C[t,h] = max over cores of bucket count
    t_e = tile_of[dst_a]
    d_e = dloc[dst_a]
    key = (owner * N_TILES + t_e) * 2 + half
    cnt = np.bincount(key, minlength=N_CORES * N_TILES * 2).reshape(
        N_CORES, N_TILES, 2
    )
    C = cnt.max(axis=0)                      # [T, 2]

    # slot layout: runs (b, h); tiles sequential, run padded to GSZ multiple
    tile_slot_base = np.zeros((N_TILES, 2), np.int64)
    runs = []          # (b, h, g0, g1, s0, s1)  group + segment ranges
    calls = []         # (b, h, g0, g1)
    segments = []      # (g, tl_local, b, t_global)
    slot_cursor = 0
    group_cursor = 0
    for b in range(N_SB):
        t_lo = b * TPB
        t_hi = min(t_lo + TPB, N_TILES)
        for h in (0, 1):
            run_s0 = slot_cursor
            run_g0 = group_cursor
            seg_s0 = len(segments)
            for t in range(t_lo, t_hi):
                tile_slot_base[t, h] = slot_cursor
                slot_cursor += int(C[t, h])
            run_slots = slot_cursor - run_s0
            run_slots_p = -(-run_slots // GSZ) * GSZ
            slot_cursor = run_s0 + run_slots_p
            n_g = run_slots_p // GSZ
            group_cursor += n_g
            g = run_g0
            while g < group_cursor:
                g1 = min(g + CALL_MAX_GROUPS, group_cursor)
                calls.append((b, h, g, g1))
                g = g1
            for gl in range(n_g):
                s0 = run_s0 + gl * GSZ
                s1 = s0 + GSZ
                for t in range(t_lo, t_hi):
                    tb = tile_slot_base[t, h]
                    te = tb + int(C[t, h])
                    if tb < s1 and te > s0:
                        segments.append((run_g0 + gl, t - t_lo, b, t))
            runs.append((b, h, run_g0, group_cursor, seg_s0, len(segments)))
    n_groups = group_cursor
    n_segs = len(segments)
    tot_slots = n_groups * GSZ

    # vectorized (group, tile) -> segment lookup
    seg_arr = np.full(n_groups * N_TILES, -1, np.int64)
    for s, (g, tl, b, t) in enumerate(segments):
        seg_arr[g * N_TILES + t] = s

    # per-core slot assignment
    idxA = np.zeros((N_CORES, tot_slots), np.int16)
    idxB = np.zeros((N_CORES, tot_slots), np.int16)
    S_val = np.zeros((N_CORES, GSZ, n_segs * TN), np.float32)
    for c in range(N_CORES):
        m = owner == c
        s_c = src_a[m]
        t_c = t_e[m]
        h_c = half[m]
        n_c = norm_a[m]
        d_c = d_e[m]
        keyth = t_c * 2 + h_c
        order = np.argsort(keyth, kind="stable")
        k_s = keyth[order]
        start_of = np.searchsorted(k_s, np.arange(N_TILES * 2))
        ranks = np.arange(k_s.size) - start_of[k_s]
        slots = tile_slot_base.reshape(-1)[k_s] + ranks
        g_of = slots // GSZ
        seg_of = seg_arr[g_of * N_TILES + t_c[order]]
        idxA[c, slots] = (s_c[order] - h_c[order] * HALF).astype(np.int16)
        rb = row_l2[s_c[order]]
        idxB[c, slots] = (rb - h_c[order] * HALF).astype(np.int16)
        S_val[c, slots % GSZ, seg_of * TN + d_c[order]] = n_c[order]

    def wrap(idx):
        out = np.empty((N_CORES, 128, tot_slots // 16), np.int16)
        for c in range(N_CORES):
            a = idx[c].reshape(tot_slots // 16, 16).T
            out[c] = np.tile(a, (8, 1))
        return out

    sched = {
        "runs": runs,
        "calls": calls,
        "segments": segments,
        "n_groups": n_groups,
        "n_segs": n_segs,
        "perm_global": perm_global,
    }
    return sched, (wrap(idxA), wrap(idxB)), S_val


def _build_program(sched, repeat=1, repeat_phase="all"):
    import concourse.bacc as bacc
    import concourse.mybir as mybir
    import concourse.tile as tile

    runs = sched["runs"]
    calls = sched["calls"]
    segments = sched["segments"]
    n_groups = sched["n_groups"]
    n_segs = sched["n_segs"]
    tot_slots = n_groups * GSZ
    f32 = mybir.dt.float32
    bf16 = mybir.dt.bfloat16

    nc = bacc.Bacc("TRN2", target_bir_lowering=False, debug=False,
                   num_devices=N_CORES,
                   dynamic_dma_scratch_size=_DMA_SCRATCH,
                   num_swdge_queues=N_QUEUES)

    d_xTf = nc.dram_tensor("xTf", [F_IN, N_NODES], bf16, kind="ExternalInput")
    d_W1 = nc.dram_tensor("W1", [F_IN, F_H1], bf16, kind="ExternalInput")
    d_W2 = nc.dram_tensor("W2", [F_H1, F_H2], f32, kind="ExternalInput")
    d_b1 = nc.dram_tensor("b1c", [F_H1, 1], f32, kind="ExternalInput")
    d_b2 = nc.dram_tensor("b2r", [128, 4 * F_H2], f32, kind="ExternalInput")
    d_S1 = nc.dram_tensor("S1", [128, n_segs * TN], bf16, kind="ExternalInput")
    d_idxA = nc.dram_tensor("idxA", [128, tot_slots // 16], mybir.dt.int16,
                            kind="ExternalInput")
    d_idxB = nc.dram_tensor("idxB", [128, tot_slots // 16], mybir.dt.int16,
                            kind="ExternalInput")
    d_out = nc.dram_tensor("out", [OWN, F_H2], f32, kind="ExternalOutput")

    rg = [list(range(N_CORES))]

    with tile.TileContext(nc) as tc:
        with (
            tc.tile_pool(name="sb", bufs=1) as sb,
            tc.tile_pool(name="sbx", bufs=2) as sbx,
            tc.tile_pool(name="sbg", bufs=GDST_BUFS) as sbg,
            tc.tile_pool(name="sbs", bufs=4) as sbs,
            tc.tile_pool(name="sbh", bufs=2) as sbh,
            tc.tile_pool(name="ps", bufs=2, space="PSUM") as ps,
            tc.tile_pool(name="dram", bufs=1, space="DRAM") as dram,
        ):
            # --- resident tiles ---
            W1_t = sb.tile([F_IN, F_H1], bf16)
            nc.sync.dma_start(W1_t[:], d_W1[:])
            W2_t = sb.tile([F_H1, F_H2], f32)
            nc.sync.dma_start(W2_t[:], d_W2[:])
            b1_t = sb.tile([F_H1, 1], f32)
            nc.sync.dma_start(b1_t[:], d_b1[:])
            b2_t = sb.tile([128, 4 * F_H2], f32)
            nc.sync.dma_start(b2_t[:], d_b2[:])
            idxA_t = sb.tile([128, tot_slots // 16], mybir.dt.int16)
            nc.sync.dma_start(idxA_t[:], d_idxA[:])
            idxB_t = sb.tile([128, tot_slots // 16], mybir.dt.int16)
            nc.sync.dma_start(idxB_t[:], d_idxB[:])
            out1T = sb.tile([F_H1, N_TILES * TN], f32)
            out2 = sb.tile([128, ((OWN + 127) // 128) * F_H2, ], f32)

            # two gatherable h1 half-tables (bf16, 256B rows); separate tiles
            # so gathers of half A don't wait on half-B writes
            h1_tabA = dram.tile([HALF, ROW1], bf16)
            h1_tabB = dram.tile([HALF, ROW1], bf16)
            h2_shard = dram.tile([OWN, F_H2], f32)
            h2_table = dram.tile([N_NODES, F_H2], f32)

            def _phase_reps(name):
                if repeat_phase == "all" or repeat_phase == name:
                    return repeat
                return 1

            for _rep in range(repeat if repeat_phase == "all" else 1):
                # --- phase A: full replicated h1 = x @ W1 -> half tables ---
                for hh, o in [(hh, o) for _ in range(_phase_reps("denseA"))
                              for hh in (0, 1)
                              for o in range(0, HALF, XCHUNK)]:
                    w = min(XCHUNK, HALF - o)
                    base = hh * HALF + o            # global node base
                    nblk = (w + 127) // 128
                    wp = nblk * 128                 # block-padded width
                    xc = sbx.tile([F_IN, XCHUNK], bf16, tag="xc",
                                  name=f"xc_{base}")
                    nc.sync.dma_start(xc[:, :w], d_xTf[:, base:base + w])
                    if wp > w:                      # zero-pad the tail block
                        nc.vector.memset(xc[:, w:wp], 0.0)
                    stg = sbh.tile([128, (XCHUNK // 128) * ROW1], bf16,
                                   tag="h1stg", name=f"stg_{base}")
                    tab = h1_tabA if hh == 0 else h1_tabB
                    # zero-fill the 96:128 pad cols the copies skip
                    nc.vector.memset(
                        stg[:, 0:nblk * ROW1].rearrange(
                            "p (c f) -> p c f", f=ROW1)[:, :, F_H1:], 0.0)
                    for j0 in range(0, nblk, DBATCH):
                        j1 = min(j0 + DBATCH, nblk)
                        p_d = ps.tile([128, DBATCH * F_H1], f32, tag="dense",
                                      name=f"pd1_{base}_{j0}")
                        for k in range(j0, j1):
                            nc.tensor.matmul(
                                out=p_d[:, (k - j0) * F_H1:(k - j0 + 1) * F_H1],
                                lhsT=xc[:, k * 128:(k + 1) * 128],
                                rhs=W1_t[:],
                                start=(k == j0), stop=(k == j1 - 1),
                            )
                        nc.scalar.copy(
                            out=stg[:, j0 * ROW1:j1 * ROW1].rearrange(
                                "p (c f) -> p c f", f=ROW1)[:, :, 0:F_H1],
                            in_=p_d[:, 0:(j1 - j0) * F_H1].rearrange(
                                "p (c f) -> p c f", f=F_H1),
                        )
                    wfull = (w // 128) * 128
                    if wfull:
                        nc.sync.dma_start(
                            tab[o:o + wfull, :].rearrange(
                                "(c p) f -> p c f", p=128),
                            stg[:, 0:(wfull // 128) * ROW1].rearrange(
                                "p (c f) -> p c f", f=ROW1),
                        )
                    if w - wfull:
                        nc.sync.dma_start(
                            tab[o + wfull:o + w, :],
                            stg[:w - wfull,
                                (wfull // 128) * ROW1:(wfull // 128 + 1) * ROW1],
                        )

                # map group index -> (call index, slot within call)
                call_of_group = {}
                for ci, (b, h, g0, g1) in enumerate(calls):
                    for g in range(g0, g1):
                        call_of_group[g] = (ci, g - g0)

                def agg_layer(tag, tabs, idx_t, d_S, s_dt, elem, alloc_cb,
                              mm_cb, out_cb, bank_of, convert=False):
                    """Gather + segment-matmul driver, two passes by half.

                    tabs = (half0_table_ap, half1_table_ap)
                    alloc_cb(b, h) -> psum tile for (superblock, pass)
                    mm_cb(p_t, gt, goff, S_t, seg_local, tl, (start, stop))
                    out_cb(b, psum_tile, w_tiles, h)  # h=0 write, h=1 accum
                    bank_of(tl) -> psum zero-region id for start/stop flags
                    convert: downcast gathered f32 tiles to bf16 (ACT copy)
                    """
                    call_tiles = [None] * len(calls)
                    order = [ci for ci, c in enumerate(calls) if c[1] == 0] + \
                            [ci for ci, c in enumerate(calls) if c[1] == 1]
                    for ci in order:
                        b, h, g0, g1 = calls[ci]
                        nblk = g1 - g0
                        gt = sbg.tile([128, nblk * elem], tabs[h].dtype,
                                      tag="gdst", name=f"g_{tag}_{g0}")
                        nslots = nblk * GSZ
                        if 'gather' not in _DBG_SKIP:
                            nc.gpsimd.dma_gather(
                                out_ap=gt[:].rearrange("p (b e) -> p b e",
                                                       e=elem),
                                in_ap=tabs[h],
                                idxs_ap=idx_t[:, g0 * GSZ // 16:
                                              g1 * GSZ // 16],
                                num_idxs=nslots,
                                num_idxs_reg=nslots,
                                elem_size=elem,
                                elem_step=elem,
                                queue_num=ci % N_QUEUES,
                            )
                        if convert:
                            gtb = sbg.tile([128, nblk * elem], bf16,
                                           tag="gconv", name=f"gb_{tag}_{g0}")
                            nc.scalar.copy(out=gtb[:], in_=gt[:])
                            call_tiles[ci] = gtb
                        else:
                            call_tiles[ci] = gt
                    # per-(pass, superblock): S chunk + segment matmuls
                    for hp in (0, 1):
                        for b, h, g0b, g1b, s0, s1 in runs:
                            if h != hp or s0 == s1:
                                continue
                            w_tiles = min(TPB, N_TILES - b * TPB)
                            S_t = sbs.tile([128, (s1 - s0) * TN], s_dt,
                                           tag="sseg",
                                           name=f"s_{tag}_{b}_{h}")
                            nc.sync.dma_start(S_t[:],
                                              d_S[:, s0 * TN:s1 * TN])
                            p_t = alloc_cb(b, h)
                            first_in_bank, last_in_bank = {}, {}
                            for si in range(s0, s1):
                                bk = bank_of(segments[si][1])
                                if bk not in first_in_bank:
                                    first_in_bank[bk] = si
                                last_in_bank[bk] = si
                            if 'mm' not in _DBG_SKIP:
                                for si in range(s0, s1):
                                    g, tl = segments[si][0], segments[si][1]
                                    bk = bank_of(tl)
                                    ci, goff = call_of_group[g]
                                    mm_cb(p_t, call_tiles[ci], goff, S_t,
                                          si - s0, tl,
                                          (first_in_bank[bk] == si,
                                           last_in_bank[bk] == si))
                                out_cb(b, p_t, w_tiles, hp)

                # L1 callbacks: h-stationary -> psum [96, TPB*TN]
                def l1_alloc(b, h):
                    return ps.tile([F_H1, TPB * TN], f32, tag="agg1",
                                   name=f"pa1_{b}_{h}")

                def l1_mm(p_t, gt, goff, S_t, sl, tl, flags):
                    start, stop = flags
                    nc.tensor.matmul(
                        out=p_t[:, tl * TN:(tl + 1) * TN],
                        lhsT=gt[:, goff * ROW1: goff * ROW1 + F_H1],
                        rhs=S_t[:, sl * TN:(sl + 1) * TN],
                        start=start, stop=stop,
                    )

                def l1_out(b, p_t, w_tiles, h):
                    w = w_tiles * TN
                    dst = out1T[:, b * TPB * TN: b * TPB * TN + w]
                    if h == 0:
                        nc.vector.tensor_scalar(
                            out=dst, in0=p_t[:, :w],
                            scalar1=b1_t[:, 0:1], scalar2=None,
                            op0=mybir.AluOpType.add,
                        )
                    else:
                        nc.vector.tensor_tensor(
                            out=dst, in0=dst, in1=p_t[:, :w],
                            op=mybir.AluOpType.add,
                        )

                if 'mm' in _DBG_SKIP:
                    nc.vector.memset(out1T[:], 0.0)
                if 'agg1' not in _DBG_SKIP:
                    for _pr in range(_phase_reps("agg1")):
                        agg_layer("l1", (h1_tabA[:], h1_tabB[:]), idxA_t,
                                  d_S1, bf16, ROW1,
                                  l1_alloc, l1_mm, l1_out, lambda tl: 0)
                else:
                    nc.vector.memset(out1T[:], 0.0)

                # --- phase D: h2_own = out1 @ W2 -> h2_shard ---
                for r0 in [r for _ in range(_phase_reps("denseD"))
                           for r in range(0, OWN, 1024)]:
                    w = min(1024, OWN - r0)
                    nblk = (w + 127) // 128
                    stg2 = sbh.tile([128, 8 * F_H2], f32, tag="h2stg",
                                    name=f"stg2_{r0}")
                    p_d = ps.tile([128, 8 * F_H2], f32, tag="dense",
                                  name=f"pd2_{r0}")
                    for k in range(nblk):
                        # out1T is N_TILES*TN = 6272 wide, so the tail block
                        # can read a full 128 cols (tile-195 pad cols)
                        nc.tensor.matmul(
                            out=p_d[:, k * F_H2:(k + 1) * F_H2],
                            lhsT=out1T[:, r0 + k * 128:r0 + (k + 1) * 128],
                            rhs=W2_t[:],
                            start=(k == 0), stop=(k == nblk - 1),
                        )
                    nc.scalar.copy(out=stg2[:, :nblk * F_H2],
                                   in_=p_d[:, :nblk * F_H2])
                    wfull = (w // 128) * 128
                    if wfull:
                        nc.sync.dma_start(
                            h2_shard[r0:r0 + wfull, :].rearrange(
                                "(c p) f -> p c f", p=128),
                            stg2[:, 0:(wfull // 128) * F_H2].rearrange(
                                "p (c f) -> p c f", f=F_H2),
                        )
                    if w - wfull:
                        nc.sync.dma_start(
                            h2_shard[r0 + wfull:r0 + w, :],
                            stg2[:w - wfull,
                                 (wfull // 128) * F_H2:(wfull // 128 + 1) * F_H2],
                        )

                # --- phase E: AllGather h2 ---
                if 'ag2' not in _DBG_SKIP:
                  for _pr in range(_phase_reps("ag2")):
                    nc.gpsimd.collective_compute(
                        "AllGather", mybir.AluOpType.bypass, replica_groups=rg,
                        ins=[h2_shard.opt()], outs=[h2_table.opt()],
                    )
                else:
                    nc.sync.dma_start(h2_table[0:OWN, :], h2_shard[:])

                # --- phase F: layer-2 aggregation (row-major out) ---
                def l2_alloc(b, h):
                    return ps.tile([TN, TPB * F_H2], f32, tag="agg2",
                                   name=f"pa2_{b}_{h}")

                def l2_mm(p_t, gt, goff, S_t, sl, tl, flags):
                    start, stop = flags
                    nc.tensor.matmul(
                        out=p_t[:, tl * F_H2:(tl + 1) * F_H2],
                        lhsT=S_t[:, sl * TN:(sl + 1) * TN],
                        rhs=gt[:, goff * F_H2:(goff + 1) * F_H2],
                        start=start, stop=stop,
                    )

                def l2_out(b, p_t, w_tiles, h):
                    # psum [32, tl*64] ; node n = b*TPB*TN + tl*TN + j
                    # -> out2 partition 32*(tl%4)+j, chunk 4*b + tl//4
                    for q in range(min(4, w_tiles)):
                        # tiles tl = 4*c' + q for c' in range(n_q)
                        n_q = (w_tiles - q + 3) // 4
                        src = p_t[:, q * F_H2:].rearrange(
                            "p (c f) -> p c f", f=F_H2)[:, 0:4 * (n_q - 1) + 1:4, :]
                        dstp = out2[q * TN:(q + 1) * TN,
                                    (4 * b) * F_H2:(4 * b + n_q) * F_H2]
                        dstr = dstp.rearrange("p (c f) -> p c f", f=F_H2)
                        if h == 0:
                            nc.vector.tensor_tensor(
                                out=dstr, in0=src,
                                in1=b2_t[q * TN:(q + 1) * TN,
                                         :n_q * F_H2].rearrange(
                                    "p (c f) -> p c f", f=F_H2),
                                op=mybir.AluOpType.add,
                            )
                        else:
                            nc.vector.tensor_tensor(
                                out=dstr, in0=dstr, in1=src,
                                op=mybir.AluOpType.add,
                            )

                if 'mm' in _DBG_SKIP:
                    nc.vector.memset(out2[:], 0.0)
                if 'agg2' not in _DBG_SKIP:
                    for _pr in range(_phase_reps("agg2")):
                        agg_layer("l2", (h2_table[:HALF, :], h2_table[HALF:, :]),
                                  idxB_t, d_S1, bf16, F_H2,
                                  l2_alloc, l2_mm, l2_out,
                                  lambda tl: tl * F_H2 * 4 // 2048,
                                  convert=True)
                else:
                    nc.vector.memset(out2[:], 0.0)

                # --- final output ---
                full = (OWN // 128) * 128        # 6144
                nc.sync.dma_start(
                    d_out[0:full, :].rearrange("(c p) f -> p c f", p=128),
                    out2[:, 0:(full // 128) * F_H2].rearrange(
                        "p (c f) -> p c f", f=F_H2),
                )
                rem = OWN - full
                if rem:
                    nc.sync.dma_start(
                        d_out[full:OWN, :],
                        out2[:rem, (full // 128) * F_H2:(full // 128 + 1) * F_H2],
                    )

    nc.compile()
    return nc


def _make_in_maps(x, W1, b1, W2, b2, S_all, idx_wrapped):
    import ml_dtypes

    idxA, idxB = idx_wrapped
    xTf = np.ascontiguousarray(
        np.asarray(x, np.float32).T.astype(ml_dtypes.bfloat16))
    b2r = np.ascontiguousarray(np.tile(np.asarray(b2, np.float32)[None, :],
                                       (128, 4)))
    in_maps = []
    for c in range(N_CORES):
        in_maps.append({
            "xTf": xTf,
            "W1": np.asarray(W1, np.float32).astype(ml_dtypes.bfloat16),
            "W2": np.asarray(W2, np.float32),
            "b1c": np.ascontiguousarray(np.asarray(b1, np.float32)[:, None]),
            "b2r": b2r,
            "S1": np.ascontiguousarray(S_all[c].astype(ml_dtypes.bfloat16)),
            "idxA": np.ascontiguousarray(idxA[c]),
            "idxB": np.ascontiguousarray(idxB[c]),
        })
    return in_maps


def _postprocess(out, sched):
    res = np.empty_like(out)
    res[sched["perm_global"]] = out
    return res


def kernel(x, edge_index, W1, b1, W2, b2):
    from concourse.bass_utils import run_bass_kernel_spmd

    ei = np.asarray(edge_index)
    src = ei[0].astype(np.int64)
    dst = ei[1].astype(np.int64)

    sched, idx_wrapped, S_all = _preprocess(src, dst)
    nc = _build_program(sched)

    in_maps = _make_in_maps(x, W1, b1, W2, b2, S_all, idx_wrapped)
    res = run_bass_kernel_spmd(nc, in_maps, core_ids=list(range(N_CORES)))
    out = np.concatenate([res.results[c]["out"] for c in range(N_CORES)], axis=0)
    return _postprocess(out.astype(np.float32), sched)


# revision 22
# speedup vs baseline: 1.1507x; 1.0500x over previous
"""2-layer GCNConv (PyG-style, normalize=True) on 8 Trainium2 NeuronCores.

Strategy (graph/data parallel, per sharding hint):
- Nodes sharded 8 ways (core c owns rows [c*6250, (c+1)*6250)); edges
  partitioned by destination-node owner.
- Weights replicated. Layer-1 dense transform (h1 = x @ W1) is computed
  REPLICATED on every core over all 50000 nodes in bf16 (PE time is
  trivial), eliminating the layer-1 AllGather; h1 is written to two local
  DRAM bf16 half-tables (src < 25000 / >= 25000) so per-edge gathers for
  half 0 start while half 1 is still being written.
- Per-edge source features fetched with dma_gather (SWDGE gather ucode);
  segment-sum by destination done on the TensorEngine as matmuls against
  host-built segment matrices S (norm coefficients baked in), accumulated
  in PSUM. Self-loop terms are folded in as extra edges with norm = 1/deg.
- Schedule: per-core balanced node->tile permutation (greedy bin-pack by
  per-half in-degree) with SHARED per-(tile,half) bucket capacities, and
  128-slot groups that span tile boundaries (matmul per (group, tile)
  segment) -- cuts slot padding from +41% to +9%. The permutation is
  undone on host after the run.
- Aggregation runs as two passes (half 0 then half 1): each pass gathers
  and accumulates its own PSUM per superblock, combined into the output
  SBUF tile by DVE (write+bias, then add), decoupling half-0 progress
  from half-1 table availability.
- Layer 2: h2 = out1 @ W2 on own rows only (f32), one AllGather builds
  the gatherable f32 h2 table (rows = permuted node order, so the L2
  gather uses its own idx table).
- deg/norm/schedule are integer-graph-structure preprocessing on host.

Layer 1 aggregation is computed transposed (gathered rows stationary,
S moving) so its output [96, own] directly feeds layer 2's dense matmul
as the stationary operand. Layer 2 aggregation is computed row-major
(S stationary) so the final output lands in row layout.
"""

import numpy as np

# problem constants (hardcoded per contract)
N_NODES = 50000
N_CORES = 8
OWN = N_NODES // N_CORES          # 6250
F_IN = 96
F_H1 = 96                         # layer-1 output width
F_H2 = 64                         # layer-2 output width
ROW1 = 128                        # h1-table row: 128 bf16 = 256 B
HALF = 25000                      # int16 gather index split point
TN = 32                           # nodes per segment tile (PSUM cols per matmul)
TPB = 16                          # node-tiles per superblock (1 PSUM bank for L1)
GSZ = 128                         # edge slots per group (= contraction dim)
CALL_MAX_GROUPS = 8               # max groups per dma_gather call (1024 slots; SWDGE ring limit)

N_TILES = (OWN + TN - 1) // TN            # 196
N_SB = (N_TILES + TPB - 1) // TPB         # 13
CAP_LAST = OWN - (N_TILES - 1) * TN       # 10
_DBG_SKIP = set()  # debug: subset of {'ag2','agg1','agg2','gather','mm'}
_DMA_SCRATCH = 65536              # SWDGE ring (SBUF B/partition): 4096 descs
GDST_BUFS = 6
N_QUEUES = 4
XCHUNK = 2048                     # xT streaming chunk (dense-1 phase)
DBATCH = 4                        # dense-1 node blocks batched per PSUM bank


def _balance(d0, d1):
    """Greedy assign OWN nodes (per-half loads d0,d1) into N_TILES tiles of
    <=TN nodes (last tile CAP_LAST), minimizing per-tile per-half max load."""
    caps = np.full(N_TILES, TN, np.int64)
    caps[-1] = CAP_LAST
    order = np.argsort(-(d0 + d1), kind="stable")
    l0 = np.zeros(N_TILES)
    l1 = np.zeros(N_TILES)
    used = np.zeros(N_TILES, np.int64)
    tile_of = np.empty(OWN, np.int64)
    dloc = np.empty(OWN, np.int64)
    for n in order:
        score = np.maximum(l0 + d0[n], l1 + d1[n])
        score[used >= caps] = np.inf
        t = int(np.argmin(score))
        tile_of[n] = t
        dloc[n] = used[t]
        used[t] += 1
        l0[t] += d0[n]
        l1[t] += d1[n]
    return tile_of, dloc


def _preprocess(src, dst):
    """Host-side integer-structure preprocessing -> shared schedule +
    per-core idx/S arrays."""
    deg = np.bincount(dst, minlength=N_NODES).astype(np.float64) + 1.0
    dinv = (1.0 / np.sqrt(deg)).astype(np.float32)

    # fold self-loops in as edges
    arange_n = np.arange(N_NODES, dtype=np.int64)
    src_a = np.concatenate([src, arange_n])
    dst_a = np.concatenate([dst, arange_n])
    norm_a = np.concatenate([
        dinv[src] * dinv[dst],
        (dinv * dinv).astype(np.float32),
    ]).astype(np.float32)

    owner = dst_a // OWN
    half = (src_a >= HALF).astype(np.int64)

    # per-dst-node per-half in-edge counts (self loops folded in)
    nodecnt = np.bincount(dst_a * 2 + half, minlength=N_NODES * 2).reshape(
        N_NODES, 2
    )

    # per-core balanced permutation
    tile_of = np.empty(N_NODES, np.int64)
    dloc = np.empty(N_NODES, np.int64)
    for c in range(N_CORES):
        lo, hi = c * OWN, (c + 1) * OWN
        t_c, d_c = _balance(nodecnt[lo:hi, 0].astype(np.float64),
                            nodecnt[lo:hi, 1].astype(np.float64))
        tile_of[lo:hi] = t_c
        dloc[lo:hi] = d_c

    p_local = tile_of * TN + dloc            # permuted local row, < OWN
    row_l2 = (np.arange(N_NODES) // OWN) * OWN + p_local
    perm_global = np.empty(N_NODES, np.int64)
    perm_global[row_l2] = np.arange(N_NODES)  # device row -> natural node

    # shared bucket capacities: C[t,h] = max over cores of bucket count
    t_e = tile_of[dst_a]
    d_e = dloc[dst_a]
    key = (owner * N_TILES + t_e) * 2 + half
    cnt = np.bincount(key, minlength=N_CORES * N_TILES * 2).reshape(
        N_CORES, N_TILES, 2
    )
    C = cnt.max(axis=0)                      # [T, 2]

    # slot layout: runs (b, h); tiles sequential, run padded to GSZ multiple
    tile_slot_base = np.zeros((N_TILES, 2), np.int64)
    runs = []          # (b, h, g0, g1, s0, s1)  group + segment ranges
    calls = []         # (b, h, g0, g1)
    segments = []      # (g, tl_local, b, t_global)
    slot_cursor = 0
    group_cursor = 0
    for b in range(N_SB):
        t_lo = b * TPB
        t_hi = min(t_lo + TPB, N_TILES)
        for h in (0, 1):
            run_s0 = slot_cursor
            run_g0 = group_cursor
            seg_s0 = len(segments)
            for t in range(t_lo, t_hi):
                tile_slot_base[t, h] = slot_cursor
                slot_cursor += int(C[t, h])
            run_slots = slot_cursor - run_s0
            run_slots_p = -(-run_slots // GSZ) * GSZ
            slot_cursor = run_s0 + run_slots_p
            n_g = run_slots_p // GSZ
            group_cursor += n_g
            g = run_g0
            while g < group_cursor:
                g1 = min(g + CALL_MAX_GROUPS, group_cursor)
                calls.append((b, h, g, g1))
                g = g1
            for gl in range(n_g):
                s0 = run_s0 + gl * GSZ
                s1 = s0 + GSZ
                for t in range(t_lo, t_hi):
                    tb = tile_slot_base[t, h]
                    te = tb + int(C[t, h])
                    if tb < s1 and te > s0:
                        segments.append((run_g0 + gl, t - t_lo, b, t))
            runs.append((b, h, run_g0, group_cursor, seg_s0, len(segments)))
    n_groups = group_cursor
    n_segs = len(segments)
    tot_slots = n_groups * GSZ

    # vectorized (group, tile) -> segment lookup
    seg_arr = np.full(n_groups * N_TILES, -1, np.int64)
    for s, (g, tl, b, t) in enumerate(segments):
        seg_arr[g * N_TILES + t] = s

    # per-core slot assignment
    idxA = np.zeros((N_CORES, tot_slots), np.int16)
    idxB = np.zeros((N_CORES, tot_slots), np.int16)
    S_val = np.zeros((N_CORES, GSZ, n_segs * TN), np.float32)
    for c in range(N_CORES):
        m = owner == c
        s_c = src_a[m]
        t_c = t_e[m]
        h_c = half[m]
        n_c = norm_a[m]
        d_c = d_e[m]
        keyth = t_c * 2 + h_c
        order = np.argsort(keyth, kind="stable")
        k_s = keyth[order]
        start_of = np.searchsorted(k_s, np.arange(N_TILES * 2))
        ranks = np.arange(k_s.size) - start_of[k_s]
        slots = tile_slot_base.reshape(-1)[k_s] + ranks
        g_of = slots // GSZ
        seg_of = seg_arr[g_of * N_TILES + t_c[order]]
        idxA[c, slots] = (s_c[order] - h_c[order] * HALF).astype(np.int16)
        rb = row_l2[s_c[order]]
        idxB[c, slots] = (rb - h_c[order] * HALF).astype(np.int16)
        S_val[c, slots % GSZ, seg_of * TN + d_c[order]] = n_c[order]

    def wrap(idx):
        out = np.empty((N_CORES, 128, tot_slots // 16), np.int16)
        for c in range(N_CORES):
            a = idx[c].reshape(tot_slots // 16, 16).T
            out[c] = np.tile(a, (8, 1))
        return out

    sched = {
        "runs": runs,
        "calls": calls,
        "segments": segments,
        "n_groups": n_groups,
        "n_segs": n_segs,
        "perm_global": perm_global,
    }
    return sched, (wrap(idxA), wrap(idxB)), S_val


def _build_program(sched, repeat=1, repeat_phase="all"):
    import concourse.bacc as bacc
    import concourse.mybir as mybir
    import concourse.tile as tile

    runs = sched["runs"]
    calls = sched["calls"]
    segments = sched["segments"]
    n_groups = sched["n_groups"]
    n_segs = sched["n_segs"]
    tot_slots = n_groups * GSZ
    f32 = mybir.dt.float32
    bf16 = mybir.dt.bfloat16

    nc = bacc.Bacc("TRN2", target_bir_lowering=False, debug=False,
                   num_devices=N_CORES,
                   dynamic_dma_scratch_size=_DMA_SCRATCH,
                   num_swdge_queues=N_QUEUES)

    d_xTf = nc.dram_tensor("xTf", [F_IN, N_NODES], bf16, kind="ExternalInput")
    d_W1 = nc.dram_tensor("W1", [F_IN, F_H1], bf16, kind="ExternalInput")
    d_W2 = nc.dram_tensor("W2", [F_H1, F_H2], f32, kind="ExternalInput")
    d_b1 = nc.dram_tensor("b1c", [F_H1, 1], f32, kind="ExternalInput")
    d_b2 = nc.dram_tensor("b2r", [128, 4 * F_H2], f32, kind="ExternalInput")
    d_S1 = nc.dram_tensor("S1", [128, n_segs * TN], bf16, kind="ExternalInput")
    d_idxA = nc.dram_tensor("idxA", [128, tot_slots // 16], mybir.dt.int16,
                            kind="ExternalInput")
    d_idxB = nc.dram_tensor("idxB", [128, tot_slots // 16], mybir.dt.int16,
                            kind="ExternalInput")
    d_out = nc.dram_tensor("out", [OWN, F_H2], f32, kind="ExternalOutput")

    rg = [list(range(N_CORES))]

    with tile.TileContext(nc) as tc:
        with (
            tc.tile_pool(name="sb", bufs=1) as sb,
            tc.tile_pool(name="sbx", bufs=2) as sbx,
            tc.tile_pool(name="sbg", bufs=GDST_BUFS) as sbg,
            tc.tile_pool(name="sbs", bufs=4) as sbs,
            tc.tile_pool(name="sbh", bufs=2) as sbh,
            tc.tile_pool(name="ps", bufs=2, space="PSUM") as ps,
            tc.tile_pool(name="dram", bufs=1, space="DRAM") as dram,
        ):
            # --- resident tiles ---
            W1_t = sb.tile([F_IN, F_H1], bf16)
            nc.sync.dma_start(W1_t[:], d_W1[:])
            W2_t = sb.tile([F_H1, F_H2], f32)
            nc.sync.dma_start(W2_t[:], d_W2[:])
            b1_t = sb.tile([F_H1, 1], f32)
            nc.sync.dma_start(b1_t[:], d_b1[:])
            b2_t = sb.tile([128, 4 * F_H2], f32)
            nc.sync.dma_start(b2_t[:], d_b2[:])
            idxA_t = sb.tile([128, tot_slots // 16], mybir.dt.int16)
            nc.sync.dma_start(idxA_t[:], d_idxA[:])
            idxB_t = sb.tile([128, tot_slots // 16], mybir.dt.int16)
            nc.sync.dma_start(idxB_t[:], d_idxB[:])
            out1T = sb.tile([F_H1, N_TILES * TN], f32)
            out2 = sb.tile([128, ((OWN + 127) // 128) * F_H2, ], f32)

            # two gatherable h1 half-tables (bf16, 256B rows); separate tiles
            # so gathers of half A don't wait on half-B writes
            h1_tabA = dram.tile([HALF, ROW1], bf16)
            h1_tabB = dram.tile([HALF, ROW1], bf16)
            h2_shard = dram.tile([OWN, F_H2], f32)
            h2_table = dram.tile([N_NODES, F_H2], f32)

            def _phase_reps(name):
                if repeat_phase == "all" or repeat_phase == name:
                    return repeat
                return 1

            for _rep in range(repeat if repeat_phase == "all" else 1):
                # --- phase A: full replicated h1 = x @ W1 -> half tables ---
                for hh, o in [(hh, o) for _ in range(_phase_reps("denseA"))
                              for hh in (0, 1)
                              for o in range(0, HALF, XCHUNK)]:
                    w = min(XCHUNK, HALF - o)
                    base = hh * HALF + o            # global node base
                    nblk = (w + 127) // 128
                    wp = nblk * 128                 # block-padded width
                    xc = sbx.tile([F_IN, XCHUNK], bf16, tag="xc",
                                  name=f"xc_{base}")
                    nc.sync.dma_start(xc[:, :w], d_xTf[:, base:base + w])
                    if wp > w:                      # zero-pad the tail block
                        nc.vector.memset(xc[:, w:wp], 0.0)
                    stg = sbh.tile([128, (XCHUNK // 128) * ROW1], bf16,
                                   tag="h1stg", name=f"stg_{base}")
                    tab = h1_tabA if hh == 0 else h1_tabB
                    # zero-fill the 96:128 pad cols the copies skip
                    nc.vector.memset(
                        stg[:, 0:nblk * ROW1].rearrange(
                            "p (c f) -> p c f", f=ROW1)[:, :, F_H1:], 0.0)
                    for j0 in range(0, nblk, DBATCH):
                        j1 = min(j0 + DBATCH, nblk)
                        p_d = ps.tile([128, DBATCH * F_H1], f32, tag="dense",
                                      name=f"pd1_{base}_{j0}")
                        for k in range(j0, j1):
                            nc.tensor.matmul(
                                out=p_d[:, (k - j0) * F_H1:(k - j0 + 1) * F_H1],
                                lhsT=xc[:, k * 128:(k + 1) * 128],
                                rhs=W1_t[:],
                                start=(k == j0), stop=(k == j1 - 1),
                            )
                        nc.scalar.copy(
                            out=stg[:, j0 * ROW1:j1 * ROW1].rearrange(
                                "p (c f) -> p c f", f=ROW1)[:, :, 0:F_H1],
                            in_=p_d[:, 0:(j1 - j0) * F_H1].rearrange(
                                "p (c f) -> p c f", f=F_H1),
                        )
                    wfull = (w // 128) * 128
                    if wfull:
                        nc.sync.dma_start(
                            tab[o:o + wfull, :].rearrange(
                                "(c p) f -> p c f", p=128),
                            stg[:, 0:(wfull // 128) * ROW1].rearrange(
                                "p (c f) -> p c f", f=ROW1),
                        )
                    if w - wfull:
                        nc.sync.dma_start(
                            tab[o + wfull:o + w, :],
                            stg[:w - wfull,
                                (wfull // 128) * ROW1:(wfull // 128 + 1) * ROW1],
                        )

                # map group index -> (call index, slot within call)
                call_of_group = {}
                for ci, (b, h, g0, g1) in enumerate(calls):
                    for g in range(g0, g1):
                        call_of_group[g] = (ci, g - g0)

                def agg_layer(tag, tabs, idx_t, d_S, s_dt, elem, alloc_cb,
                              mm_cb, out_cb, bank_of, convert=False):
                    """Gather + segment-matmul driver, two passes by half.

                    tabs = (half0_table_ap, half1_table_ap)
                    alloc_cb(b, h) -> psum tile for (superblock, pass)
                    mm_cb(p_t, gt, goff, S_t, seg_local, tl, (start, stop))
                    out_cb(b, psum_tile, w_tiles, h)  # h=0 write, h=1 accum
                    bank_of(tl) -> psum zero-region id for start/stop flags
                    convert: downcast gathered f32 tiles to bf16 (ACT copy)
                    """
                    call_tiles = [None] * len(calls)
                    order = [ci for ci, c in enumerate(calls) if c[1] == 0] + \
                            [ci for ci, c in enumerate(calls) if c[1] == 1]
                    for ci in order:
                        b, h, g0, g1 = calls[ci]
                        nblk = g1 - g0
                        gt = sbg.tile([128, nblk * elem], tabs[h].dtype,
                                      tag="gdst", name=f"g_{tag}_{g0}")
                        nslots = nblk * GSZ
                        if 'gather' not in _DBG_SKIP:
                            nc.gpsimd.dma_gather(
                                out_ap=gt[:].rearrange("p (b e) -> p b e",
                                                       e=elem),
                                in_ap=tabs[h],
                                idxs_ap=idx_t[:, g0 * GSZ // 16:
                                              g1 * GSZ // 16],
                                num_idxs=nslots,
                                num_idxs_reg=nslots,
                                elem_size=elem,
                                elem_step=elem,
                                queue_num=ci % N_QUEUES,
                            )
                        if convert:
                            gtb = sbg.tile([128, nblk * elem], bf16,
                                           tag="gconv", name=f"gb_{tag}_{g0}")
                            nc.scalar.copy(out=gtb[:], in_=gt[:])
                            call_tiles[ci] = gtb
                        else:
                            call_tiles[ci] = gt
                    # per-(pass, superblock): S chunk + segment matmuls
                    for hp in (0, 1):
                        for b, h, g0b, g1b, s0, s1 in runs:
                            if h != hp or s0 == s1:
                                continue
                            w_tiles = min(TPB, N_TILES - b * TPB)
                            S_t = sbs.tile([128, (s1 - s0) * TN], s_dt,
                                           tag="sseg",
                                           name=f"s_{tag}_{b}_{h}")
                            nc.sync.dma_start(S_t[:],
                                              d_S[:, s0 * TN:s1 * TN])
                            p_t = alloc_cb(b, h)
                            first_in_bank, last_in_bank = {}, {}
                            for si in range(s0, s1):
                                bk = bank_of(segments[si][1])
                                if bk not in first_in_bank:
                                    first_in_bank[bk] = si
                                last_in_bank[bk] = si
                            if 'mm' not in _DBG_SKIP:
                                for si in range(s0, s1):
                                    g, tl = segments[si][0], segments[si][1]
                                    bk = bank_of(tl)
                                    ci, goff = call_of_group[g]
                                    mm_cb(p_t, call_tiles[ci], goff, S_t,
                                          si - s0, tl,
                                          (first_in_bank[bk] == si,
                                           last_in_bank[bk] == si))
                                out_cb(b, p_t, w_tiles, hp)

                # L1 callbacks: h-stationary -> psum [96, TPB*TN]
                def l1_alloc(b, h):
                    return ps.tile([F_H1, TPB * TN], f32, tag="agg1",
                                   name=f"pa1_{b}_{h}")

                def l1_mm(p_t, gt, goff, S_t, sl, tl, flags):
                    start, stop = flags
                    nc.tensor.matmul(
                        out=p_t[:, tl * TN:(tl + 1) * TN],
                        lhsT=gt[:, goff * ROW1: goff * ROW1 + F_H1],
                        rhs=S_t[:, sl * TN:(sl + 1) * TN],
                        start=start, stop=stop,
                    )

                def l1_out(b, p_t, w_tiles, h):
                    w = w_tiles * TN
                    dst = out1T[:, b * TPB * TN: b * TPB * TN + w]
                    if h == 0:
                        nc.vector.tensor_scalar(
                            out=dst, in0=p_t[:, :w],
                            scalar1=b1_t[:, 0:1], scalar2=None,
                            op0=mybir.AluOpType.add,
                        )
                    else:
                        nc.vector.tensor_tensor(
                            out=dst, in0=dst, in1=p_t[:, :w],
                            op=mybir.AluOpType.add,
                        )

                if 'mm' in _DBG_SKIP:
                    nc.vector.memset(out1T[:], 0.0)
                if 'agg1' not in _DBG_SKIP:
                    for _pr in range(_phase_reps("agg1")):
                        agg_layer("l1", (h1_tabA[:], h1_tabB[:]), idxA_t,
                                  d_S1, bf16, ROW1,
                                  l1_alloc, l1_mm, l1_out, lambda tl: 0)
                else:
                    nc.vector.memset(out1T[:], 0.0)

                # --- phase D: h2_own = out1 @ W2 -> h2_shard ---
                for r0 in [r for _ in range(_phase_reps("denseD"))
                           for r in range(0, OWN, 1024)]:
                    w = min(1024, OWN - r0)
                    nblk = (w + 127) // 128
                    stg2 = sbh.tile([128, 8 * F_H2], f32, tag="h2stg",
                                    name=f"stg2_{r0}")
                    p_d = ps.tile([128, 8 * F_H2], f32, tag="dense",
                                  name=f"pd2_{r0}")
                    for k in range(nblk):
                        # out1T is N_TILES*TN = 6272 wide, so the tail block
                        # can read a full 128 cols (tile-195 pad cols)
                        nc.tensor.matmul(
                            out=p_d[:, k * F_H2:(k + 1) * F_H2],
                            lhsT=out1T[:, r0 + k * 128:r0 + (k + 1) * 128],
                            rhs=W2_t[:],
                            start=(k == 0), stop=(k == nblk - 1),
                        )
                    nc.scalar.copy(out=stg2[:, :nblk * F_H2],
                                   in_=p_d[:, :nblk * F_H2])
                    wfull = (w // 128) * 128
                    if wfull:
                        nc.sync.dma_start(
                            h2_shard[r0:r0 + wfull, :].rearrange(
                                "(c p) f -> p c f", p=128),
                            stg2[:, 0:(wfull // 128) * F_H2].rearrange(
                                "p (c f) -> p c f", f=F_H2),
                        )
                    if w - wfull:
                        nc.sync.dma_start(
                            h2_shard[r0 + wfull:r0 + w, :],
                            stg2[:w - wfull,
                                 (wfull // 128) * F_H2:(wfull // 128 + 1) * F_H2],
                        )

                # --- phase E: AllGather h2 ---
                if 'ag2' not in _DBG_SKIP:
                  for _pr in range(_phase_reps("ag2")):
                    nc.gpsimd.collective_compute(
                        "AllGather", mybir.AluOpType.bypass, replica_groups=rg,
                        ins=[h2_shard.opt()], outs=[h2_table.opt()],
                    )
                else:
                    nc.sync.dma_start(h2_table[0:OWN, :], h2_shard[:])

                # --- phase F: layer-2 aggregation (row-major out) ---
                def l2_alloc(b, h):
                    return ps.tile([TN, TPB * F_H2], f32, tag="agg2",
                                   name=f"pa2_{b}_{h}")

                def l2_mm(p_t, gt, goff, S_t, sl, tl, flags):
                    start, stop = flags
                    nc.tensor.matmul(
                        out=p_t[:, tl * F_H2:(tl + 1) * F_H2],
                        lhsT=S_t[:, sl * TN:(sl + 1) * TN],
                        rhs=gt[:, goff * F_H2:(goff + 1) * F_H2],
                        start=start, stop=stop,
                    )

                def l2_out(b, p_t, w_tiles, h):
                    # psum [32, tl*64] ; node n = b*TPB*TN + tl*TN + j
                    # -> out2 partition 32*(tl%4)+j, chunk 4*b + tl//4
                    for q in range(min(4, w_tiles)):
                        # tiles tl = 4*c' + q for c' in range(n_q)
                        n_q = (w_tiles - q + 3) // 4
                        src = p_t[:, q * F_H2:].rearrange(
                            "p (c f) -> p c f", f=F_H2)[:, 0:4 * (n_q - 1) + 1:4, :]
                        dstp = out2[q * TN:(q + 1) * TN,
                                    (4 * b) * F_H2:(4 * b + n_q) * F_H2]
                        dstr = dstp.rearrange("p (c f) -> p c f", f=F_H2)
                        if h == 0:
                            nc.vector.tensor_tensor(
                                out=dstr, in0=src,
                                in1=b2_t[q * TN:(q + 1) * TN,
                                         :n_q * F_H2].rearrange(
                                    "p (c f) -> p c f", f=F_H2),
                                op=mybir.AluOpType.add,
                            )
                        else:
                            nc.vector.tensor_tensor(
                                out=dstr, in0=dstr, in1=src,
                                op=mybir.AluOpType.add,
                            )

                if 'mm' in _DBG_SKIP:
                    nc.vector.memset(out2[:], 0.0)
                if 'agg2' not in _DBG_SKIP:
                    for _pr in range(_phase_reps("agg2")):
                        agg_layer("l2", (h2_table[:HALF, :], h2_table[HALF:, :]),
                                  idxB_t, d_S1, bf16, F_H2,
                                  l2_alloc, l2_mm, l2_out,
                                  lambda tl: tl * F_H2 * 4 // 2048,
                                  convert=True)
                else:
                    nc.vector.memset(out2[:], 0.0)

                # --- final output ---
                full = (OWN // 128) * 128        # 6144
                nc.sync.dma_start(
                    d_out[0:full, :].rearrange("(c p) f -> p c f", p=128),
                    out2[:, 0:(full // 128) * F_H2].rearrange(
                        "p (c f) -> p c f", f=F_H2),
                )
                rem = OWN - full
                if rem:
                    nc.sync.dma_start(
                        d_out[full:OWN, :],
                        out2[:rem, (full // 128) * F_H2:(full // 128 + 1) * F_H2],
                    )

    nc.compile()
    return nc


def _make_in_maps(x, W1, b1, W2, b2, S_all, idx_wrapped):
    import ml_dtypes

    idxA, idxB = idx_wrapped
    xTf = np.ascontiguousarray(
        np.asarray(x, np.float32).T.astype(ml_dtypes.bfloat16))
    b2r = np.ascontiguousarray(np.tile(np.asarray(b2, np.float32)[None, :],
                                       (128, 4)))
    in_maps = []
    for c in range(N_CORES):
        in_maps.append({
            "xTf": xTf,
            "W1": np.asarray(W1, np.float32).astype(ml_dtypes.bfloat16),
            "W2": np.asarray(W2, np.float32),
            "b1c": np.ascontiguousarray(np.asarray(b1, np.float32)[:, None]),
            "b2r": b2r,
            "S1": np.ascontiguousarray(S_all[c].astype(ml_dtypes.bfloat16)),
            "idxA": np.ascontiguousarray(idxA[c]),
            "idxB": np.ascontiguousarray(idxB[c]),
        })
    return in_maps


def _postprocess(out, sched):
    res = np.empty_like(out)
    res[sched["perm_global"]] = out
    return res


def kernel(x, edge_index, W1, b1, W2, b2):
    from concourse.bass_utils import run_bass_kernel_spmd

    ei = np.asarray(edge_index)
    src = ei[0].astype(np.int64)
    dst = ei[1].astype(np.int64)

    sched, idx_wrapped, S_all = _preprocess(src, dst)
    nc = _build_program(sched)

    in_maps = _make_in_maps(x, W1, b1, W2, b2, S_all, idx_wrapped)
    res = run_bass_kernel_spmd(nc, in_maps, core_ids=list(range(N_CORES)))
    out = np.concatenate([res.results[c]["out"] for c in range(N_CORES)], axis=0)
    return _postprocess(out.astype(np.float32), sched)


# revision 23
# speedup vs baseline: 1.3282x; 1.1542x over previous
"""2-layer GCNConv (PyG-style, normalize=True) on 8 Trainium2 NeuronCores.

Strategy (graph/data parallel, per sharding hint):
- Nodes sharded 8 ways (core c owns rows [c*6250, (c+1)*6250)); edges
  partitioned by destination-node owner.
- Weights replicated. Layer-1 dense transform (h1 = x @ W1) is computed
  REPLICATED on every core over all 50000 nodes in bf16 (PE time is
  trivial), eliminating the layer-1 AllGather; h1 is written to two local
  DRAM bf16 half-tables (src < 25000 / >= 25000) so per-edge gathers for
  half 0 start while half 1 is still being written.
- Per-edge source features fetched with dma_gather (SWDGE gather ucode);
  segment-sum by destination done on the TensorEngine as matmuls against
  host-built segment matrices S (norm coefficients baked in), accumulated
  in PSUM. Self-loop terms are folded in as extra edges with norm = 1/deg.
- Schedule: per-core balanced node->tile permutation (greedy bin-pack by
  per-half in-degree) with SHARED per-(tile,half) bucket capacities, and
  128-slot groups that span tile boundaries (matmul per (group, tile)
  segment) -- cuts slot padding from +41% to +9%. The permutation is
  undone on host after the run.
- Aggregation runs as two passes (half 0 then half 1): each pass gathers
  and accumulates its own PSUM per superblock, combined into the output
  SBUF tile by DVE (write+bias, then add), decoupling half-0 progress
  from half-1 table availability.
- Layer 2: h2 = out1 @ W2 on own rows only (f32), one AllGather builds
  the gatherable f32 h2 table (rows = permuted node order, so the L2
  gather uses its own idx table).
- deg/norm/schedule are integer-graph-structure preprocessing on host.

Layer 1 aggregation is computed transposed (gathered rows stationary,
S moving) so its output [96, own] directly feeds layer 2's dense matmul
as the stationary operand. Layer 2 aggregation is computed row-major
(S stationary) so the final output lands in row layout.
"""

import numpy as np

# problem constants (hardcoded per contract)
N_NODES = 50000
N_CORES = 8
OWN = N_NODES // N_CORES          # 6250
F_IN = 96
F_H1 = 96                         # layer-1 output width
F_H2 = 64                         # layer-2 output width
ROW1 = 128                        # h1-table row: 128 bf16 = 256 B
HALF = 25000                      # int16 gather index split point
TN = 32                           # nodes per segment tile (PSUM cols per matmul)
TPB = 16                          # node-tiles per superblock (1 PSUM bank for L1)
GSZ = 128                         # edge slots per group (= contraction dim)
CALL_MAX_GROUPS = 4               # max groups per dma_gather call (512 slots)

N_TILES = (OWN + TN - 1) // TN            # 196
N_SB = (N_TILES + TPB - 1) // TPB         # 13
CAP_LAST = OWN - (N_TILES - 1) * TN       # 10
_DBG_SKIP = set()  # debug: subset of {'ag2','agg1','agg2','gather','mm'}
_DMA_SCRATCH = 65536              # SWDGE ring (SBUF B/partition): 4096 descs
GDST_BUFS = 6
N_QUEUES = 4
XCHUNK = 2048                     # xT streaming chunk (dense-1 phase)
DBATCH = 4                        # dense-1 node blocks batched per PSUM bank


def _balance(d0, d1):
    """Greedy assign OWN nodes (per-half loads d0,d1) into N_TILES tiles of
    <=TN nodes (last tile CAP_LAST), minimizing per-tile per-half max load."""
    caps = np.full(N_TILES, TN, np.int64)
    caps[-1] = CAP_LAST
    order = np.argsort(-(d0 + d1), kind="stable")
    l0 = np.zeros(N_TILES)
    l1 = np.zeros(N_TILES)
    used = np.zeros(N_TILES, np.int64)
    tile_of = np.empty(OWN, np.int64)
    dloc = np.empty(OWN, np.int64)
    for n in order:
        score = np.maximum(l0 + d0[n], l1 + d1[n])
        score[used >= caps] = np.inf
        t = int(np.argmin(score))
        tile_of[n] = t
        dloc[n] = used[t]
        used[t] += 1
        l0[t] += d0[n]
        l1[t] += d1[n]
    return tile_of, dloc


def _preprocess(src, dst):
    """Host-side integer-structure preprocessing -> shared schedule +
    per-core idx/S arrays."""
    deg = np.bincount(dst, minlength=N_NODES).astype(np.float64) + 1.0
    dinv = (1.0 / np.sqrt(deg)).astype(np.float32)

    # fold self-loops in as edges
    arange_n = np.arange(N_NODES, dtype=np.int64)
    src_a = np.concatenate([src, arange_n])
    dst_a = np.concatenate([dst, arange_n])
    norm_a = np.concatenate([
        dinv[src] * dinv[dst],
        (dinv * dinv).astype(np.float32),
    ]).astype(np.float32)

    owner = dst_a // OWN
    half = (src_a >= HALF).astype(np.int64)

    # per-dst-node per-half in-edge counts (self loops folded in)
    nodecnt = np.bincount(dst_a * 2 + half, minlength=N_NODES * 2).reshape(
        N_NODES, 2
    )

    # per-core balanced permutation
    tile_of = np.empty(N_NODES, np.int64)
    dloc = np.empty(N_NODES, np.int64)
    for c in range(N_CORES):
        lo, hi = c * OWN, (c + 1) * OWN
        t_c, d_c = _balance(nodecnt[lo:hi, 0].astype(np.float64),
                            nodecnt[lo:hi, 1].astype(np.float64))
        tile_of[lo:hi] = t_c
        dloc[lo:hi] = d_c

    p_local = tile_of * TN + dloc            # permuted local row, < OWN
    row_l2 = (np.arange(N_NODES) // OWN) * OWN + p_local
    perm_global = np.empty(N_NODES, np.int64)
    perm_global[row_l2] = np.arange(N_NODES)  # device row -> natural node

    # shared bucket capacities: C[t,h] = max over cores of bucket count
    t_e = tile_of[dst_a]
    d_e = dloc[dst_a]
    key = (owner * N_TILES + t_e) * 2 + half
    cnt = np.bincount(key, minlength=N_CORES * N_TILES * 2).reshape(
        N_CORES, N_TILES, 2
    )
    C = cnt.max(axis=0)                      # [T, 2]

    # slot layout: runs (b, h); tiles sequential, run padded to GSZ multiple
    tile_slot_base = np.zeros((N_TILES, 2), np.int64)
    runs = []          # (b, h, g0, g1, s0, s1)  group + segment ranges
    calls = []         # (b, h, g0, g1)
    segments = []      # (g, tl_local, b, t_global)
    slot_cursor = 0
    group_cursor = 0
    for b in range(N_SB):
        t_lo = b * TPB
        t_hi = min(t_lo + TPB, N_TILES)
        for h in (0, 1):
            run_s0 = slot_cursor
            run_g0 = group_cursor
            seg_s0 = len(segments)
            for t in range(t_lo, t_hi):
                tile_slot_base[t, h] = slot_cursor
                slot_cursor += int(C[t, h])
            run_slots = slot_cursor - run_s0
            run_slots_p = -(-run_slots // GSZ) * GSZ
            slot_cursor = run_s0 + run_slots_p
            n_g = run_slots_p // GSZ
            group_cursor += n_g
            g = run_g0
            while g < group_cursor:
                g1 = min(g + CALL_MAX_GROUPS, group_cursor)
                calls.append((b, h, g, g1))
                g = g1
            for gl in range(n_g):
                s0 = run_s0 + gl * GSZ
                s1 = s0 + GSZ
                for t in range(t_lo, t_hi):
                    tb = tile_slot_base[t, h]
                    te = tb + int(C[t, h])
                    if tb < s1 and te > s0:
                        segments.append((run_g0 + gl, t - t_lo, b, t))
            runs.append((b, h, run_g0, group_cursor, seg_s0, len(segments)))
    n_groups = group_cursor
    n_segs = len(segments)
    tot_slots = n_groups * GSZ

    # vectorized (group, tile) -> segment lookup
    seg_arr = np.full(n_groups * N_TILES, -1, np.int64)
    for s, (g, tl, b, t) in enumerate(segments):
        seg_arr[g * N_TILES + t] = s

    # per-core slot assignment
    idxA = np.zeros((N_CORES, tot_slots), np.int16)
    idxB = np.zeros((N_CORES, tot_slots), np.int16)
    S_val = np.zeros((N_CORES, GSZ, n_segs * TN), np.float32)
    for c in range(N_CORES):
        m = owner == c
        s_c = src_a[m]
        t_c = t_e[m]
        h_c = half[m]
        n_c = norm_a[m]
        d_c = d_e[m]
        keyth = t_c * 2 + h_c
        order = np.argsort(keyth, kind="stable")
        k_s = keyth[order]
        start_of = np.searchsorted(k_s, np.arange(N_TILES * 2))
        ranks = np.arange(k_s.size) - start_of[k_s]
        slots = tile_slot_base.reshape(-1)[k_s] + ranks
        g_of = slots // GSZ
        seg_of = seg_arr[g_of * N_TILES + t_c[order]]
        idxA[c, slots] = (s_c[order] - h_c[order] * HALF).astype(np.int16)
        rb = row_l2[s_c[order]]
        idxB[c, slots] = (rb - h_c[order] * HALF).astype(np.int16)
        S_val[c, slots % GSZ, seg_of * TN + d_c[order]] = n_c[order]

    def wrap(idx):
        out = np.empty((N_CORES, 128, tot_slots // 16), np.int16)
        for c in range(N_CORES):
            a = idx[c].reshape(tot_slots // 16, 16).T
            out[c] = np.tile(a, (8, 1))
        return out

    sched = {
        "runs": runs,
        "calls": calls,
        "segments": segments,
        "n_groups": n_groups,
        "n_segs": n_segs,
        "perm_global": perm_global,
    }
    return sched, (wrap(idxA), wrap(idxB)), S_val


def _build_program(sched, repeat=1, repeat_phase="all"):
    import concourse.bacc as bacc
    import concourse.mybir as mybir
    import concourse.tile as tile

    runs = sched["runs"]
    calls = sched["calls"]
    segments = sched["segments"]
    n_groups = sched["n_groups"]
    n_segs = sched["n_segs"]
    tot_slots = n_groups * GSZ
    f32 = mybir.dt.float32
    bf16 = mybir.dt.bfloat16

    nc = bacc.Bacc("TRN2", target_bir_lowering=False, debug=False,
                   num_devices=N_CORES,
                   dynamic_dma_scratch_size=_DMA_SCRATCH,
                   num_swdge_queues=N_QUEUES)

    d_xTf = nc.dram_tensor("xTf", [F_IN, N_NODES], bf16, kind="ExternalInput")
    d_W1 = nc.dram_tensor("W1", [F_IN, F_H1], bf16, kind="ExternalInput")
    d_W2 = nc.dram_tensor("W2", [F_H1, F_H2], f32, kind="ExternalInput")
    d_b1 = nc.dram_tensor("b1c", [F_H1, 1], f32, kind="ExternalInput")
    d_b2 = nc.dram_tensor("b2r", [128, 4 * F_H2], f32, kind="ExternalInput")
    d_S1 = nc.dram_tensor("S1", [128, n_segs * TN], bf16, kind="ExternalInput")
    d_idxA = nc.dram_tensor("idxA", [128, tot_slots // 16], mybir.dt.int16,
                            kind="ExternalInput")
    d_idxB = nc.dram_tensor("idxB", [128, tot_slots // 16], mybir.dt.int16,
                            kind="ExternalInput")
    d_out = nc.dram_tensor("out", [OWN, F_H2], f32, kind="ExternalOutput")

    rg = [list(range(N_CORES))]

    with tile.TileContext(nc) as tc:
        with (
            tc.tile_pool(name="sb", bufs=1) as sb,
            tc.tile_pool(name="sbx", bufs=2) as sbx,
            tc.tile_pool(name="sbg", bufs=GDST_BUFS) as sbg,
            tc.tile_pool(name="sbs", bufs=4) as sbs,
            tc.tile_pool(name="sbh", bufs=2) as sbh,
            tc.tile_pool(name="ps", bufs=2, space="PSUM") as ps,
            tc.tile_pool(name="dram", bufs=1, space="DRAM") as dram,
        ):
            # --- resident tiles ---
            W1_t = sb.tile([F_IN, F_H1], bf16)
            nc.sync.dma_start(W1_t[:], d_W1[:])
            W2_t = sb.tile([F_H1, F_H2], f32)
            nc.sync.dma_start(W2_t[:], d_W2[:])
            b1_t = sb.tile([F_H1, 1], f32)
            nc.sync.dma_start(b1_t[:], d_b1[:])
            b2_t = sb.tile([128, 4 * F_H2], f32)
            nc.sync.dma_start(b2_t[:], d_b2[:])
            idxA_t = sb.tile([128, tot_slots // 16], mybir.dt.int16)
            nc.sync.dma_start(idxA_t[:], d_idxA[:])
            idxB_t = sb.tile([128, tot_slots // 16], mybir.dt.int16)
            nc.sync.dma_start(idxB_t[:], d_idxB[:])
            out1T = sb.tile([F_H1, N_TILES * TN], f32)
            out2 = sb.tile([128, ((OWN + 127) // 128) * F_H2, ], f32)

            # two gatherable h1 half-tables (bf16, 256B rows); separate tiles
            # so gathers of half A don't wait on half-B writes
            h1_tabA = dram.tile([HALF, ROW1], bf16)
            h1_tabB = dram.tile([HALF, ROW1], bf16)
            h2_shard = dram.tile([OWN, F_H2], f32)
            h2_table = dram.tile([N_NODES, F_H2], f32)

            def _phase_reps(name):
                if repeat_phase == "all" or repeat_phase == name:
                    return repeat
                return 1

            for _rep in range(repeat if repeat_phase == "all" else 1):
                # --- phase A: full replicated h1 = x @ W1 -> half tables ---
                for hh, o in [(hh, o) for _ in range(_phase_reps("denseA"))
                              for hh in (0, 1)
                              for o in range(0, HALF, XCHUNK)]:
                    w = min(XCHUNK, HALF - o)
                    base = hh * HALF + o            # global node base
                    nblk = (w + 127) // 128
                    wp = nblk * 128                 # block-padded width
                    xc = sbx.tile([F_IN, XCHUNK], bf16, tag="xc",
                                  name=f"xc_{base}")
                    nc.sync.dma_start(xc[:, :w], d_xTf[:, base:base + w])
                    if wp > w:                      # zero-pad the tail block
                        nc.vector.memset(xc[:, w:wp], 0.0)
                    stg = sbh.tile([128, (XCHUNK // 128) * ROW1], bf16,
                                   tag="h1stg", name=f"stg_{base}")
                    tab = h1_tabA if hh == 0 else h1_tabB
                    # zero-fill the 96:128 pad cols the copies skip
                    nc.vector.memset(
                        stg[:, 0:nblk * ROW1].rearrange(
                            "p (c f) -> p c f", f=ROW1)[:, :, F_H1:], 0.0)
                    for j0 in range(0, nblk, DBATCH):
                        j1 = min(j0 + DBATCH, nblk)
                        p_d = ps.tile([128, DBATCH * F_H1], f32, tag="dense",
                                      name=f"pd1_{base}_{j0}")
                        for k in range(j0, j1):
                            nc.tensor.matmul(
                                out=p_d[:, (k - j0) * F_H1:(k - j0 + 1) * F_H1],
                                lhsT=xc[:, k * 128:(k + 1) * 128],
                                rhs=W1_t[:],
                                start=(k == j0), stop=(k == j1 - 1),
                            )
                        nc.scalar.copy(
                            out=stg[:, j0 * ROW1:j1 * ROW1].rearrange(
                                "p (c f) -> p c f", f=ROW1)[:, :, 0:F_H1],
                            in_=p_d[:, 0:(j1 - j0) * F_H1].rearrange(
                                "p (c f) -> p c f", f=F_H1),
                        )
                    wfull = (w // 128) * 128
                    if wfull:
                        nc.sync.dma_start(
                            tab[o:o + wfull, :].rearrange(
                                "(c p) f -> p c f", p=128),
                            stg[:, 0:(wfull // 128) * ROW1].rearrange(
                                "p (c f) -> p c f", f=ROW1),
                        )
                    if w - wfull:
                        nc.sync.dma_start(
                            tab[o + wfull:o + w, :],
                            stg[:w - wfull,
                                (wfull // 128) * ROW1:(wfull // 128 + 1) * ROW1],
                        )

                # map group index -> (call index, slot within call)
                call_of_group = {}
                for ci, (b, h, g0, g1) in enumerate(calls):
                    for g in range(g0, g1):
                        call_of_group[g] = (ci, g - g0)

                def agg_layer(tag, tabs, idx_t, d_S, s_dt, elem, alloc_cb,
                              mm_cb, out_cb, bank_of, convert=False):
                    """Gather + segment-matmul driver, two passes by half.

                    tabs = (half0_table_ap, half1_table_ap)
                    alloc_cb(b, h) -> psum tile for (superblock, pass)
                    mm_cb(p_t, gt, goff, S_t, seg_local, tl, (start, stop))
                    out_cb(b, psum_tile, w_tiles, h)  # h=0 write, h=1 accum
                    bank_of(tl) -> psum zero-region id for start/stop flags
                    convert: downcast gathered f32 tiles to bf16 (ACT copy)
                    """
                    call_tiles = [None] * len(calls)
                    order = [ci for ci, c in enumerate(calls) if c[1] == 0] + \
                            [ci for ci, c in enumerate(calls) if c[1] == 1]
                    for ci in order:
                        b, h, g0, g1 = calls[ci]
                        nblk = g1 - g0
                        gt = sbg.tile([128, nblk * elem], tabs[h].dtype,
                                      tag="gdst", name=f"g_{tag}_{g0}")
                        nslots = nblk * GSZ
                        if 'gather' not in _DBG_SKIP:
                            nc.gpsimd.dma_gather(
                                out_ap=gt[:].rearrange("p (b e) -> p b e",
                                                       e=elem),
                                in_ap=tabs[h],
                                idxs_ap=idx_t[:, g0 * GSZ // 16:
                                              g1 * GSZ // 16],
                                num_idxs=nslots,
                                num_idxs_reg=nslots,
                                elem_size=elem,
                                elem_step=elem,
                                queue_num=ci % N_QUEUES,
                            )
                        if convert:
                            gtb = sbg.tile([128, nblk * elem], bf16,
                                           tag="gconv", name=f"gb_{tag}_{g0}")
                            nc.scalar.copy(out=gtb[:], in_=gt[:])
                            call_tiles[ci] = gtb
                        else:
                            call_tiles[ci] = gt
                    # per-(pass, superblock): S chunk + segment matmuls
                    for hp in (0, 1):
                        for b, h, g0b, g1b, s0, s1 in runs:
                            if h != hp or s0 == s1:
                                continue
                            w_tiles = min(TPB, N_TILES - b * TPB)
                            S_t = sbs.tile([128, (s1 - s0) * TN], s_dt,
                                           tag="sseg",
                                           name=f"s_{tag}_{b}_{h}")
                            nc.sync.dma_start(S_t[:],
                                              d_S[:, s0 * TN:s1 * TN])
                            p_t = alloc_cb(b, h)
                            first_in_bank, last_in_bank = {}, {}
                            for si in range(s0, s1):
                                bk = bank_of(segments[si][1])
                                if bk not in first_in_bank:
                                    first_in_bank[bk] = si
                                last_in_bank[bk] = si
                            if 'mm' not in _DBG_SKIP:
                                for si in range(s0, s1):
                                    g, tl = segments[si][0], segments[si][1]
                                    bk = bank_of(tl)
                                    ci, goff = call_of_group[g]
                                    mm_cb(p_t, call_tiles[ci], goff, S_t,
                                          si - s0, tl,
                                          (first_in_bank[bk] == si,
                                           last_in_bank[bk] == si))
                                out_cb(b, p_t, w_tiles, hp)

                # L1 callbacks: h-stationary -> psum [96, TPB*TN]
                def l1_alloc(b, h):
                    return ps.tile([F_H1, TPB * TN], f32, tag="agg1",
                                   name=f"pa1_{b}_{h}")

                def l1_mm(p_t, gt, goff, S_t, sl, tl, flags):
                    start, stop = flags
                    nc.tensor.matmul(
                        out=p_t[:, tl * TN:(tl + 1) * TN],
                        lhsT=gt[:, goff * ROW1: goff * ROW1 + F_H1],
                        rhs=S_t[:, sl * TN:(sl + 1) * TN],
                        start=start, stop=stop,
                    )

                def l1_out(b, p_t, w_tiles, h):
                    w = w_tiles * TN
                    dst = out1T[:, b * TPB * TN: b * TPB * TN + w]
                    if h == 0:
                        nc.vector.tensor_scalar(
                            out=dst, in0=p_t[:, :w],
                            scalar1=b1_t[:, 0:1], scalar2=None,
                            op0=mybir.AluOpType.add,
                        )
                    else:
                        nc.vector.tensor_tensor(
                            out=dst, in0=dst, in1=p_t[:, :w],
                            op=mybir.AluOpType.add,
                        )

                if 'mm' in _DBG_SKIP:
                    nc.vector.memset(out1T[:], 0.0)
                if 'agg1' not in _DBG_SKIP:
                    for _pr in range(_phase_reps("agg1")):
                        agg_layer("l1", (h1_tabA[:], h1_tabB[:]), idxA_t,
                                  d_S1, bf16, ROW1,
                                  l1_alloc, l1_mm, l1_out, lambda tl: 0)
                else:
                    nc.vector.memset(out1T[:], 0.0)

                # --- phase D: h2_own = out1 @ W2 -> h2_shard ---
                for r0 in [r for _ in range(_phase_reps("denseD"))
                           for r in range(0, OWN, 1024)]:
                    w = min(1024, OWN - r0)
                    nblk = (w + 127) // 128
                    stg2 = sbh.tile([128, 8 * F_H2], f32, tag="h2stg",
                                    name=f"stg2_{r0}")
                    p_d = ps.tile([128, 8 * F_H2], f32, tag="dense",
                                  name=f"pd2_{r0}")
                    for k in range(nblk):
                        # out1T is N_TILES*TN = 6272 wide, so the tail block
                        # can read a full 128 cols (tile-195 pad cols)
                        nc.tensor.matmul(
                            out=p_d[:, k * F_H2:(k + 1) * F_H2],
                            lhsT=out1T[:, r0 + k * 128:r0 + (k + 1) * 128],
                            rhs=W2_t[:],
                            start=(k == 0), stop=(k == nblk - 1),
                        )
                    nc.scalar.copy(out=stg2[:, :nblk * F_H2],
                                   in_=p_d[:, :nblk * F_H2])
                    wfull = (w // 128) * 128
                    if wfull:
                        nc.sync.dma_start(
                            h2_shard[r0:r0 + wfull, :].rearrange(
                                "(c p) f -> p c f", p=128),
                            stg2[:, 0:(wfull // 128) * F_H2].rearrange(
                                "p (c f) -> p c f", f=F_H2),
                        )
                    if w - wfull:
                        nc.sync.dma_start(
                            h2_shard[r0 + wfull:r0 + w, :],
                            stg2[:w - wfull,
                                 (wfull // 128) * F_H2:(wfull // 128 + 1) * F_H2],
                        )

                # --- phase E: AllGather h2 ---
                if 'ag2' not in _DBG_SKIP:
                  for _pr in range(_phase_reps("ag2")):
                    nc.gpsimd.collective_compute(
                        "AllGather", mybir.AluOpType.bypass, replica_groups=rg,
                        ins=[h2_shard.opt()], outs=[h2_table.opt()],
                    )
                else:
                    nc.sync.dma_start(h2_table[0:OWN, :], h2_shard[:])

                # --- phase F: layer-2 aggregation (row-major out) ---
                def l2_alloc(b, h):
                    return ps.tile([TN, TPB * F_H2], f32, tag="agg2",
                                   name=f"pa2_{b}_{h}")

                def l2_mm(p_t, gt, goff, S_t, sl, tl, flags):
                    start, stop = flags
                    nc.tensor.matmul(
                        out=p_t[:, tl * F_H2:(tl + 1) * F_H2],
                        lhsT=S_t[:, sl * TN:(sl + 1) * TN],
                        rhs=gt[:, goff * F_H2:(goff + 1) * F_H2],
                        start=start, stop=stop,
                    )

                def l2_out(b, p_t, w_tiles, h):
                    # psum [32, tl*64] ; node n = b*TPB*TN + tl*TN + j
                    # -> out2 partition 32*(tl%4)+j, chunk 4*b + tl//4
                    for q in range(min(4, w_tiles)):
                        # tiles tl = 4*c' + q for c' in range(n_q)
                        n_q = (w_tiles - q + 3) // 4
                        src = p_t[:, q * F_H2:].rearrange(
                            "p (c f) -> p c f", f=F_H2)[:, 0:4 * (n_q - 1) + 1:4, :]
                        dstp = out2[q * TN:(q + 1) * TN,
                                    (4 * b) * F_H2:(4 * b + n_q) * F_H2]
                        dstr = dstp.rearrange("p (c f) -> p c f", f=F_H2)
                        if h == 0:
                            nc.vector.tensor_tensor(
                                out=dstr, in0=src,
                                in1=b2_t[q * TN:(q + 1) * TN,
                                         :n_q * F_H2].rearrange(
                                    "p (c f) -> p c f", f=F_H2),
                                op=mybir.AluOpType.add,
                            )
                        else:
                            nc.vector.tensor_tensor(
                                out=dstr, in0=dstr, in1=src,
                                op=mybir.AluOpType.add,
                            )

                if 'mm' in _DBG_SKIP:
                    nc.vector.memset(out2[:], 0.0)
                if 'agg2' not in _DBG_SKIP:
                    for _pr in range(_phase_reps("agg2")):
                        agg_layer("l2", (h2_table[:HALF, :], h2_table[HALF:, :]),
                                  idxB_t, d_S1, bf16, F_H2,
                                  l2_alloc, l2_mm, l2_out,
                                  lambda tl: tl * F_H2 * 4 // 2048,
                                  convert=True)
                else:
                    nc.vector.memset(out2[:], 0.0)

                # --- final output ---
                full = (OWN // 128) * 128        # 6144
                nc.sync.dma_start(
                    d_out[0:full, :].rearrange("(c p) f -> p c f", p=128),
                    out2[:, 0:(full // 128) * F_H2].rearrange(
                        "p (c f) -> p c f", f=F_H2),
                )
                rem = OWN - full
                if rem:
                    nc.sync.dma_start(
                        d_out[full:OWN, :],
                        out2[:rem, (full // 128) * F_H2:(full // 128 + 1) * F_H2],
                    )

    nc.compile()
    return nc


def _make_in_maps(x, W1, b1, W2, b2, S_all, idx_wrapped):
    import ml_dtypes

    idxA, idxB = idx_wrapped
    xTf = np.ascontiguousarray(
        np.asarray(x, np.float32).T.astype(ml_dtypes.bfloat16))
    b2r = np.ascontiguousarray(np.tile(np.asarray(b2, np.float32)[None, :],
                                       (128, 4)))
    in_maps = []
    for c in range(N_CORES):
        in_maps.append({
            "xTf": xTf,
            "W1": np.asarray(W1, np.float32).astype(ml_dtypes.bfloat16),
            "W2": np.asarray(W2, np.float32),
            "b1c": np.ascontiguousarray(np.asarray(b1, np.float32)[:, None]),
            "b2r": b2r,
            "S1": np.ascontiguousarray(S_all[c].astype(ml_dtypes.bfloat16)),
            "idxA": np.ascontiguousarray(idxA[c]),
            "idxB": np.ascontiguousarray(idxB[c]),
        })
    return in_maps


def _postprocess(out, sched):
    res = np.empty_like(out)
    res[sched["perm_global"]] = out
    return res


def kernel(x, edge_index, W1, b1, W2, b2):
    from concourse.bass_utils import run_bass_kernel_spmd

    ei = np.asarray(edge_index)
    src = ei[0].astype(np.int64)
    dst = ei[1].astype(np.int64)

    sched, idx_wrapped, S_all = _preprocess(src, dst)
    nc = _build_program(sched)

    in_maps = _make_in_maps(x, W1, b1, W2, b2, S_all, idx_wrapped)
    res = run_bass_kernel_spmd(nc, in_maps, core_ids=list(range(N_CORES)))
    out = np.concatenate([res.results[c]["out"] for c in range(N_CORES)], axis=0)
    return _postprocess(out.astype(np.float32), sched)
